# revision 37
# baseline (speedup 1.0000x reference)
"""GQA (32 q heads / 8 kv heads, head_dim 64, causal, QK-RMSNorm + RoPE) on 8 TRN2 cores.

Sharding: data-parallel over batch (2) x tensor-parallel over heads (4):
each core handles one batch element, 8 query heads, 2 kv heads, and produces
a partial output (its heads' slice of the Wo contraction); the host sums the
4 partials per batch element.

v3 (from v2 baseline):
- startup: critical-path DMA order (wv, wk, x-tb0 first; consts/trig/wq after),
  bf16 trig tables, bf16 output partials (host accumulates in f32).
- phase 2: PSUM o-accumulators evacuated to SBUF by DVE immediately after the
  AV accumulation finishes (frees the PSUM bank in ~0.6us instead of ~5us so
  the next block's AV never stalls); softmax normalize (denom Ln/Exp + GpSimd
  broadcast + DVE mult) runs from the SBUF copy and is issued LATE (interleaved
  after the next block's first exp) so the Scalar engine never delays the
  exp stream the PE is waiting on.
- causal trim: the second diagonal key-block of every (qh, g) attention block
  only computes the upper query half (256 fewer score/exp/AV columns).
- output projection drained one dc-chunk per kc step as PE filler.
"""

import numpy as np
import ml_dtypes

import concourse.bass as bass
import concourse.mybir as mybir
import concourse.tile as tile
from concourse import bacc
from concourse.bass_utils import run_bass_kernel_spmd

# Keep Ln and Exp in one activation table (natural_log_exp_and_others) so the
# ACT engine never ping-pongs table loads between them: strip exp/ln from every
# other set so the table-load pass has a single candidate for both.
import concourse.hw_specs as _hw_specs

_orig_get_tables = _hw_specs.get_activation_tables


def _patched_get_tables(arch):
    _AF = mybir.ActivationFunctionType
    tabs = dict(_orig_get_tables(arch))
    out = {}
    for name, fset in tabs.items():
        if name == "natural_log_exp_and_others":
            out[name] = set(fset)
        else:
            out[name] = set(fset) - {_AF.Exp, _AF.Ln}
    return out


_hw_specs.get_activation_tables = _patched_get_tables
bacc.get_activation_tables = _patched_get_tables


# Problem config (hardcoded per contract)
B, T, D = 2, 2048, 2048
H, KV, HD = 32, 8, 64
GROUPS = H // KV
THETA = 10000.0
SCALE = 1.0 / np.sqrt(HD)
EPS = 1e-6

# Per-core sharding
HQL = H // 4          # 8 local q heads
KVL = KV // 4         # 2 local kv heads (= groups per core)
FQ = HQL * HD         # 512
FKV = KVL * HD        # 128

# Tiling
P = 128
TB = 512              # token block (phase 1)
TQ = 256              # query sub-block (phase 2)
NTB = T // TB         # 4
NDC = D // P          # 16 contraction chunks
NKC = T // P          # 16 key chunks
NQC = FQ // P         # 4 q-proj chunks (2 heads each)

f32 = mybir.dt.float32
bf16 = mybir.dt.bfloat16
AF = mybir.ActivationFunctionType
ALU = mybir.AluOpType


def _build_nc():
    nc = bacc.Bacc("TRN2", target_bir_lowering=False, debug=False, num_devices=8)

    eps_t = nc.alloc_sbuf_tensor("const-f32-eps", [128, 1], f32)
    nc.gpsimd.memset(eps_t.ap(), EPS)
    nc.const_aps.aps[(f32, EPS)] = eps_t.ap()
    nc.all_engine_barrier()

    xT_d = nc.dram_tensor("xT", [D, T], bf16, kind="ExternalInput")
    wq_d = nc.dram_tensor("wq", [P, NDC, FQ], bf16, kind="ExternalInput")
    wk_d = nc.dram_tensor("wk", [P, NDC, FKV], bf16, kind="ExternalInput")
    wv_d = nc.dram_tensor("wv", [P, NDC, FKV], bf16, kind="ExternalInput")
    wo_d = nc.dram_tensor("wo", [P, NQC, D], bf16, kind="ExternalInput")
    cosq_d = nc.dram_tensor("cosq", [P, T], bf16, kind="ExternalInput")
    cosk_d = nc.dram_tensor("cosk", [P, T], bf16, kind="ExternalInput")
    sin_d = nc.dram_tensor("sin", [P, T], bf16, kind="ExternalInput")
    rqT_d = nc.dram_tensor("rqT", [P, P], bf16, kind="ExternalInput")
    rkT_d = nc.dram_tensor("rkT", [P, P], bf16, kind="ExternalInput")
    hsel_d = nc.dram_tensor("hsel", [P, 2], bf16, kind="ExternalInput")
    hexp_d = nc.dram_tensor("hexp", [2, P], bf16, kind="ExternalInput")
    masks_d = nc.dram_tensor("masks", [P, 2, GROUPS, TQ], bf16, kind="ExternalInput")
    ident_d = nc.dram_tensor("ident", [P, P], bf16, kind="ExternalInput")
    outT_d = nc.dram_tensor("outT", [D, T], bf16, kind="ExternalOutput")
    # DRAM bounce buffers: softmax denominators spread to 128 partitions so
    # the reciprocal ACT call is 8 columns instead of 1024
    dsc_d = nc.dram_tensor("dsc", [8, 1024], bf16, kind="Internal")
    dsc2_d = nc.dram_tensor("dsc2", [8, 1024], bf16, kind="Internal")

    with tile.TileContext(nc) as tc:
        with (
            tc.tile_pool(name="wpool", bufs=1) as wpool,
            tc.tile_pool(name="cpool", bufs=1) as cpool,
            tc.tile_pool(name="kvpool", bufs=1) as kvpool,
            tc.tile_pool(name="xpool", bufs=2) as xpool,
            tc.tile_pool(name="trig", bufs=2) as trig,
            tc.tile_pool(name="bpool", bufs=2) as bpool,
            tc.tile_pool(name="epool", bufs=6) as epool,
            tc.tile_pool(name="outp", bufs=3) as outp,
            tc.tile_pool(name="opool", bufs=3) as opool,
        ):
            # ---- persistent weights / constants ----
            wq_sb = wpool.tile([P, NDC, FQ], bf16)
            wk_sb = wpool.tile([P, NDC, FKV], bf16)
            wv_sb = wpool.tile([P, NDC, FKV], bf16)
            wo_sb = wpool.tile([P, NQC, D], bf16)
            # critical path first: V/K weights in chunks interleaved with the
            # first x chunks so the first V matmul starts as early as possible

            rqT_sb = cpool.tile([P, P], bf16)
            rkT_sb = cpool.tile([P, P], bf16)
            hsel_sb = cpool.tile([P, 2], bf16)
            hexp_sb = cpool.tile([2, P], bf16)
            masks_sb = cpool.tile([P, 2, GROUPS, TQ], bf16)
            ident_sb = cpool.tile([P, P], bf16)

            # K^T per group on partition halves; V [key, kc, g, hd+ones];
            # Q packed [64g+hd partitions, head-in-group slot, token]
            ktf = kvpool.tile([P, T], bf16)
            v_sb = kvpool.tile([P, NKC, KVL, 66], bf16)
            qg = kvpool.tile([P, GROUPS, T], bf16)
            ones_bc = nc.const_aps.tensor(1.0, (P, NKC, KVL, 66), f32)
            nc.vector.tensor_copy(v_sb[:], ones_bc)

            # ---------------- Phase 1: projections + QK norm/rope ----------
            with (
                tc.tile_pool(name="psA", bufs=6, space="PSUM") as psA,
                tc.tile_pool(name="psB", bufs=2, space="PSUM") as psB,
            ):
                def load_x(tb_l):
                    t = xpool.tile([P, NDC, TB], bf16, tag="x")
                    for dc in range(NDC):
                        nc.sync.dma_start(
                            t[:, dc, :],
                            xT_d[dc * P:(dc + 1) * P,
                                 tb_l * TB:(tb_l + 1) * TB])
                    return t

                xpre = xpool.tile([P, NDC, TB], bf16, tag="x")
                for dc in range(NDC):
                    if dc < 4:
                        nc.sync.dma_start(wv_sb[:, 4 * dc:4 * dc + 4, :],
                                          wv_d[:, 4 * dc:4 * dc + 4, :])
                    elif dc in (4, 8):
                        h = (dc - 4) // 4
                        nc.sync.dma_start(wk_sb[:, 8 * h:8 * h + 8, :],
                                          wk_d[:, 8 * h:8 * h + 8, :])
                    nc.sync.dma_start(xpre[:, dc, :],
                                      xT_d[dc * P:(dc + 1) * P, 0:TB])
                for tb in range(NTB):
                    tbs = slice(tb * TB, (tb + 1) * TB)
                    xtb = xpre

                    if tb == 0:
                        # non-critical constants behind the tb0 x chunks
                        nc.sync.dma_start(ident_sb[:], ident_d[:])
                        nc.sync.dma_start(hsel_sb[:], hsel_d[:])

                    cq_t = trig.tile([P, TB], bf16, tag="cq")
                    ck_t = trig.tile([P, TB], bf16, tag="ck")
                    sn_t = trig.tile([P, TB], bf16, tag="sn")
                    nc.sync.dma_start(cq_t[:], cosq_d[:, tbs])
                    nc.sync.dma_start(ck_t[:], cosk_d[:, tbs])
                    nc.sync.dma_start(sn_t[:], sin_d[:, tbs])

                    if tb == 0:
                        nc.sync.dma_start(rqT_sb[:], rqT_d[:])
                        nc.sync.dma_start(rkT_sb[:], rkT_d[:])
                        nc.sync.dma_start(hexp_sb[:], hexp_d[:])
                        # wq quarters interleaved with the tb1 x prefetch so
                        # neither the tb0 Q projection nor the tb1 V/K
                        # projection waits on a monolithic transfer
                        xpre = xpool.tile([P, NDC, TB], bf16, tag="x")
                        for q4 in range(4):
                            nc.sync.dma_start(wq_sb[:, 4 * q4:4 * q4 + 4, :],
                                              wq_d[:, 4 * q4:4 * q4 + 4, :])
                            for dc in range(4 * q4, 4 * q4 + 4):
                                nc.sync.dma_start(
                                    xpre[:, dc, :],
                                    xT_d[dc * P:(dc + 1) * P, TB:2 * TB])
                        nc.sync.dma_start(masks_sb[:], masks_d[:])
                    elif tb == 1:
                        nc.sync.dma_start(wo_sb[:], wo_d[:])
                        xpre = load_x(2)
                    elif tb == 2:
                        xpre = load_x(3)

                    # projections: V, K, then 4 Q chunks
                    vps = psA.tile([P, TB], f32, tag="big", name="vps")
                    for dc in range(NDC):
                        nc.tensor.matmul(vps[:], wv_sb[:, dc, :], xtb[:, dc, :],
                                         start=dc == 0, stop=dc == NDC - 1)
                    vt = bpool.tile([P, TB], bf16, tag="vt")
                    nc.scalar.copy(vt[:], vps[:])
                    # V transpose on the PE (bf16), both groups per 128-chunk
                    for st4 in range(TB // P):
                        kc = tb * (TB // P) + st4
                        tp = psB.tile([P, P], bf16, tag="small", name="tp")
                        nc.tensor.transpose(tp[:], vt[:, st4 * P:(st4 + 1) * P],
                                            ident_sb[:])
                        nc.vector.tensor_copy(v_sb[:, kc, 0, 0:64], tp[:, 0:64])
                        nc.vector.tensor_copy(v_sb[:, kc, 1, 0:64], tp[:, 64:P])

                    kps = psA.tile([P, TB], f32, tag="big", name="kps")
                    for dc in range(NDC):
                        nc.tensor.matmul(kps[:], wk_sb[:, dc, :], xtb[:, dc, :],
                                         start=dc == 0, stop=dc == NDC - 1)

                    qps = [psA.tile([P, TB], f32, tag="big", name=f"qps{c}")
                           for c in range(NQC)]
                    for dc in range(NDC):
                        for c in range(NQC):
                            nc.tensor.matmul(qps[c][:],
                                             wq_sb[:, dc, c * P:(c + 1) * P],
                                             xtb[:, dc, :],
                                             start=dc == 0, stop=dc == NDC - 1)

                    # pass 1: bf16 copies + per-token sum of squares
                    chunks = qps + [kps]
                    qsb = []
                    ss_sb = bpool.tile([2, NQC + 1, TB], f32, tag="ss_sb", bufs=1)
                    for ci, cps in enumerate(chunks):
                        qs_t = bpool.tile([P, TB], bf16, tag=f"qsb{ci}")
                        nc.scalar.copy(qs_t[:], cps[:])
                        qsb.append(qs_t)
                        sq = bpool.tile([P, TB], bf16, tag="sq")
                        nc.vector.tensor_tensor(sq[:], qs_t[:], qs_t[:], ALU.mult)
                        ssp = psB.tile([2, TB], f32, tag="small", name="ssp")
                        nc.tensor.matmul(ssp[:], hsel_sb[:], sq[:],
                                         start=True, stop=True)
                        nc.scalar.copy(ss_sb[:, ci, :], ssp[:])

                    # per-chunk rsqrt: rr = exp(-0.5 * ln(ss/HD + eps));
                    # split per ci so pass 2 of ci=0 starts without waiting
                    # the full batch through the ACT queue
                    rr = bpool.tile([2, NQC + 1, TB], bf16, tag="rr", bufs=1)
                    for ci in range(NQC + 1):
                        lnb = bpool.tile([2, TB], f32, tag="lnb")
                        nc.scalar.activation(lnb[:], ss_sb[:, ci, :], AF.Ln,
                                             bias=EPS, scale=1.0 / HD)
                        nc.scalar.activation(rr[:, ci, :], lnb[:], AF.Exp,
                                             scale=-0.5)

                    # pass 2: rope + apply rsqrt, write qg / ktf
                    for ci in range(NQC + 1):
                        is_k = ci == NQC
                        rT = rkT_sb if is_k else rqT_sb
                        ct = ck_t if is_k else cq_t
                        bc = psB.tile([P, TB], f32, tag="small", name="bc")
                        nc.tensor.matmul(bc[:], hexp_sb[:], rr[:, ci, :],
                                         start=True, stop=True)
                        rot = psB.tile([P, TB], f32, tag="small", name="rot")
                        nc.tensor.matmul(rot[:], rT[:], qsb[ci][:],
                                         start=True, stop=True)
                        m1 = bpool.tile([P, TB], bf16, tag="m1")
                        nc.vector.tensor_tensor(m1[:], qsb[ci][:], ct[:], ALU.mult)
                        m2 = bpool.tile([P, TB], bf16, tag="m2")
                        nc.vector.tensor_tensor(m2[:], rot[:], sn_t[:], ALU.mult)
                        s12 = bpool.tile([P, TB], bf16, tag="s12")
                        nc.vector.tensor_tensor(s12[:], m1[:], m2[:], ALU.add)
                        if not is_k:
                            g = ci // 2
                            j0 = 2 * (ci % 2)
                            gp = slice(64 * g, 64 * g + 64)
                            nc.vector.tensor_tensor(qg[gp, j0, tbs],
                                                    s12[0:64], bc[0:64], ALU.mult)
                            nc.vector.tensor_tensor(qg[gp, j0 + 1, tbs],
                                                    s12[64:P], bc[64:P], ALU.mult)
                        else:
                            nc.vector.tensor_tensor(ktf[0:64, tbs],
                                                    s12[0:64], bc[0:64], ALU.mult)
                            nc.vector.tensor_tensor(ktf[64:P, tbs],
                                                    s12[64:P], bc[64:P], ALU.mult)

            # ---------------- Phase 2: attention + output projection --------
            with (
                tc.tile_pool(name="psSP", bufs=2, space="PSUM") as psSP,
                tc.tile_pool(name="psO", bufs=2, space="PSUM") as psO,
                tc.tile_pool(name="psM", bufs=2, space="PSUM") as psM,
            ):
                pending = []       # queued output-projection thunks (PE filler)
                norm2 = []         # deferred normalize mults (DVE)
                nblk = [0]         # block counter for DRAM bounce slots

                def queue_E(tb_e, orhs_e, dc2_range, half=None, spread=False):
                    ts0 = tb_e * TB if half is None else tb_e * TB + half * TQ
                    w = TB if half is None else TQ
                    cs = slice(0, TB) if half is None else slice(half * TQ,
                                                                 (half + 1) * TQ)
                    for dc2 in dc2_range:
                        def th(dc2=dc2):
                            acc = psM.tile([P, w], f32, tag="m", name="acc")
                            for j in range(GROUPS):
                                nc.tensor.matmul(acc[:],
                                                 wo_sb[:, j, dc2 * P:(dc2 + 1) * P],
                                                 orhs_e[:, j, cs],
                                                 start=j == 0, stop=j == GROUPS - 1)
                            ob = outp.tile([P, w], bf16, tag="ob")
                            nc.vector.tensor_copy(ob[:], acc[:])
                            # spread the descriptor generation over idle
                            # engine DMA rings on the critical tail
                            eng = (nc.sync, nc.scalar, nc.gpsimd)[
                                dc2 % 3] if spread else nc.sync
                            eng.dma_start(
                                outT_d[dc2 * P:(dc2 + 1) * P, ts0:ts0 + w], ob[:])
                        pending.append(th)

                def drain(n=1):
                    for _ in range(n):
                        if pending:
                            pending.pop(0)()

                for tb in range(NTB):
                    orhs = bpool.tile([P, GROUPS, TB], bf16, tag="orhs")
                    last = tb == NTB - 1
                    blocks = [(qh, g) for qh in range(2) for g in range(KVL)]
                    for bi, (qh, g) in enumerate(blocks):
                        gp = slice(64 * g, 64 * g + 64)
                        qbase = tb * TB + qh * TQ
                        qs = slice(qbase, qbase + TQ)
                        qsl = slice(qh * TQ, (qh + 1) * TQ)
                        nkc = qbase // P + 2
                        o01 = psO.tile([65, 2, TQ], f32, tag="o", name="o01")
                        o23 = psO.tile([65, 2, TQ], f32, tag="o", name="o23")
                        es_l = [None] * nkc
                        trim_l = [False] * nkc
                        # software pipeline: AV(kc) trails exp(kc) by one step
                        for kc in range(nkc + 1):
                            if kc < nkc:
                                trim = kc == nkc - 1  # 2nd diagonal block:
                                # queries 0..127 of this TQ fully masked
                                trim_l[kc] = trim
                                if trim:
                                    qv = slice(qbase + TQ // 2, qbase + TQ)
                                    hh = slice(0, TQ // 2)
                                    sps = psSP.tile([P, GROUPS, TQ], f32,
                                                    tag="sps")
                                    nc.tensor.matmul(
                                        sps[:, 0:2, hh],
                                        ktf[gp, kc * P:(kc + 1) * P],
                                        qg[gp, 0:2, qv], start=True, stop=True)
                                    nc.tensor.matmul(
                                        sps[:, 2:4, hh],
                                        ktf[gp, kc * P:(kc + 1) * P],
                                        qg[gp, 2:4, qv], start=True, stop=True)
                                    es = epool.tile([P, GROUPS, TQ], bf16,
                                                    tag="es")
                                    nc.scalar.activation(es[:, :, hh],
                                                         sps[:, :, hh], AF.Exp,
                                                         scale=float(SCALE))
                                    nc.vector.tensor_tensor(
                                        es[:, :, hh], es[:, :, hh],
                                        masks_sb[:, 0, :, 0:TQ // 2], ALU.mult)
                                else:
                                    sps = psSP.tile([P, GROUPS, TQ], f32,
                                                    tag="sps")
                                    nc.tensor.matmul(
                                        sps[:, 0:2, :],
                                        ktf[gp, kc * P:(kc + 1) * P],
                                        qg[gp, 0:2, qs], start=True, stop=True)
                                    nc.tensor.matmul(
                                        sps[:, 2:4, :],
                                        ktf[gp, kc * P:(kc + 1) * P],
                                        qg[gp, 2:4, qs], start=True, stop=True)
                                    es = epool.tile([P, GROUPS, TQ], bf16,
                                                    tag="es")
                                    nc.scalar.activation(es[:], sps[:], AF.Exp,
                                                         scale=float(SCALE))
                                    if kc == nkc - 2:
                                        # 1st diagonal block: triangle mask
                                        nc.vector.tensor_tensor(
                                            es[:], es[:],
                                            masks_sb[:, 0, :, :], ALU.mult)
                                es_l[kc] = es
                            # outproj filler between score(kc) and AV(kc-1)
                            # keeps the PE busy while exp(kc-1) finishes
                            drain()
                            if kc == 2 and nkc >= 6:
                                # long block: flush deferred normalize mults
                                # here (denominator chain has settled, and the
                                # DVE is idle until this block's mask mults)
                                while norm2:
                                    norm2.pop(0)()
                            if kc >= 1:
                                kp = kc - 1
                                st = kp == 0
                                sp = kp == nkc - 1
                                if trim_l[kp]:
                                    hq = slice(TQ // 2, TQ)
                                    hh = slice(0, TQ // 2)
                                    nc.tensor.matmul(
                                        o01[:, :, hq], v_sb[:, kp, g, 0:65],
                                        es_l[kp][:, 0:2, hh], start=st, stop=sp)
                                    nc.tensor.matmul(
                                        o23[:, :, hq], v_sb[:, kp, g, 0:65],
                                        es_l[kp][:, 2:4, hh], start=st, stop=sp)
                                else:
                                    nc.tensor.matmul(
                                        o01[:], v_sb[:, kp, g, 0:65],
                                        es_l[kp][:, 0:2, :], start=st, stop=sp)
                                    nc.tensor.matmul(
                                        o23[:], v_sb[:, kp, g, 0:65],
                                        es_l[kp][:, 2:4, :], start=st, stop=sp)
                        # evacuate PSUM accumulators to SBUF immediately (DVE)
                        # so the next block's AV reuses the banks without
                        # waiting for the normalize chain
                        o_sb = opool.tile([65, 2, 2, TQ], bf16, tag="osb")
                        nc.vector.tensor_copy(o_sb[:, 0, :, :], o01[:])
                        nc.vector.tensor_copy(o_sb[:, 1, :, :], o23[:])
                        while norm2:
                            norm2.pop(0)()

                        final = last and bi == len(blocks) - 1
                        if final:
                            # tail latency matters: recip straight off the
                            # denom row, broadcast on the (now idle) PE
                            dln = bpool.tile([1, 2, 2, TQ], f32, tag="dln")
                            nc.scalar.activation(dln[:], o_sb[64:65, :, :, :],
                                                 AF.Ln)
                            den = bpool.tile([1, 2, 2, TQ], bf16, tag="den")
                            nc.scalar.activation(den[:], dln[:], AF.Exp,
                                                 scale=-1.0)
                            for h in range(2):
                                bch = psM.tile([64, 2, TQ], f32, tag="m",
                                               name="bch")
                                nc.tensor.matmul(bch[:], hexp_sb[0:1, 0:64],
                                                 den[0:1, h, :, :],
                                                 start=True, stop=True)
                                nc.vector.tensor_tensor(
                                    orhs[gp, 2 * h:2 * h + 2, qsl],
                                    o_sb[0:64, h, :, :], bch[:], ALU.mult)
                        else:
                            # denom recip: bounce through DRAM to spread the
                            # 1024 values over 128 partitions so the ACT
                            # passes cost 8 columns instead of 1024
                            slot = nblk[0] % 8
                            nblk[0] += 1
                            nc.sync.dma_start(dsc_d[slot:slot + 1, :],
                                              o_sb[64:65, :, :, :])
                            dsp = bpool.tile([P, 8], bf16, tag="dsp")
                            nc.sync.dma_start(
                                dsp[:],
                                dsc_d[slot:slot + 1, :].rearrange(
                                    "o (p c) -> (o p) c", p=P, c=8))
                            dl8 = bpool.tile([P, 8], f32, tag="dl8")
                            nc.scalar.activation(dl8[:], dsp[:], AF.Ln)
                            dr8 = bpool.tile([P, 8], bf16, tag="dr8")
                            nc.scalar.activation(dr8[:], dl8[:], AF.Exp,
                                                 scale=-1.0)
                            nc.sync.dma_start(
                                dsc2_d[slot:slot + 1, :].rearrange(
                                    "o (p c) -> (o p) c", p=P, c=8), dr8[:])
                            drow = bpool.tile([1, 2, 2, TQ], bf16, tag="drow")
                            nc.sync.dma_start(drow[:], dsc2_d[slot:slot + 1, :])
                            bc2 = bpool.tile([64, 2, 2, TQ], bf16, tag="bc2")
                            nc.gpsimd.partition_broadcast(bc2[:], drow[:])

                            # the normalize mults go on DVE but DEFERRED one
                            # block, so they sit behind the next block's
                            # evacuation in the DVE queue and never delay it
                            # while the denominator DMA chain is in flight
                            def n2(o_sb=o_sb, bc2=bc2, gp=gp, qsl=qsl,
                                   orhs=orhs):
                                nc.vector.tensor_tensor(
                                    orhs[gp, 0:2, qsl], o_sb[0:64, 0, :, :],
                                    bc2[:, 0, :, :], ALU.mult)
                                nc.vector.tensor_tensor(
                                    orhs[gp, 2:4, qsl], o_sb[0:64, 1, :, :],
                                    bc2[:, 1, :, :], ALU.mult)
                            norm2.append(n2)

                        # last tb only: queue the first-half output projection
                        # before the final block (there is no later work to
                        # drain it against); all other tbs queue both halves
                        # at tb end so a drain never pops before its orhs is
                        # normalized
                        if bi == 2 and last:
                            queue_E(tb, orhs, range(NDC), half=0, spread=True)
                    while norm2:
                        norm2.pop(0)()
                    if not last:
                        queue_E(tb, orhs, range(NDC), half=0)
                    queue_E(tb, orhs, range(NDC), half=1, spread=last)
                drain(len(pending))
                drain(len(pending))

    nc.compile()
    return nc


_NC_CACHE = None


def _get_nc():
    global _NC_CACHE
    if _NC_CACHE is None:
        _NC_CACHE = _build_nc()
    return _NC_CACHE


def _host_constants(q_scale, k_scale):
    pos = np.arange(T, dtype=np.float64)
    invf = 1.0 / (THETA ** (np.arange(0, HD, 2, dtype=np.float64) / HD))  # (32,)
    ang = pos[:, None] * invf[None, :]                                    # (T, 32)
    c = np.cos(ang)
    s = np.sin(ang)
    pidx = np.arange(P) % 32
    hidx = np.arange(P) % HD
    cosq = (c[:, pidx].T * q_scale[hidx][:, None]).astype(ml_dtypes.bfloat16)
    cosk = (c[:, pidx].T * k_scale[hidx][:, None]).astype(ml_dtypes.bfloat16)
    sin = s[:, pidx].T.astype(ml_dtypes.bfloat16)

    def rmat(scale):
        R = np.zeros((HD, HD), dtype=np.float64)
        for i in range(32):
            R[i, i + 32] = -scale[i + 32]
            R[i + 32, i] = scale[i]
        M = np.kron(np.eye(2), R)
        return np.ascontiguousarray(M.T).astype(ml_dtypes.bfloat16)

    hsel = np.zeros((P, 2), dtype=np.float32)
    hsel[0:64, 0] = 1.0
    hsel[64:P, 1] = 1.0
    hexp = np.ascontiguousarray(hsel.T).astype(ml_dtypes.bfloat16)
    hsel = hsel.astype(ml_dtypes.bfloat16)

    # masks[p, i, j, f] = (f >= p + 128*i), replicated over the 4 head slots
    pp = np.arange(P)[:, None]
    ff = np.arange(TQ)[None, :]
    masks = np.zeros((P, 2, GROUPS, TQ), dtype=np.float32)
    for i in range(2):
        m = (ff >= pp + P * i).astype(np.float32)
        for j in range(GROUPS):
            masks[:, i, j, :] = m
    masks = masks.astype(ml_dtypes.bfloat16)
    ident = np.eye(P, dtype=ml_dtypes.bfloat16)

    return cosq, cosk, sin, rmat(q_scale), rmat(k_scale), hsel, hexp, masks, ident


def _run(inputs, trace=False):
    x = np.asarray(inputs["x"], dtype=np.float32)
    Wq = np.asarray(inputs["Wq"], dtype=np.float32)
    Wk = np.asarray(inputs["Wk"], dtype=np.float32)
    Wv = np.asarray(inputs["Wv"], dtype=np.float32)
    Wo = np.asarray(inputs["Wo"], dtype=np.float32)
    q_scale = np.asarray(inputs["q_scale"], dtype=np.float64)
    k_scale = np.asarray(inputs["k_scale"], dtype=np.float64)

    cosq, cosk, sin, rqT, rkT, hsel, hexp, masks, ident = _host_constants(
        q_scale, k_scale)

    in_maps = []
    for cid in range(8):
        b = cid // 4
        r = cid % 4
        # Wo rows (g, head j within group, hd) -> [64g+hd partitions, j slots]
        wo_loc = Wo[r * FQ:(r + 1) * FQ, :].reshape(KVL, GROUPS, HD, D)
        wo_loc = np.ascontiguousarray(
            wo_loc.transpose(0, 2, 1, 3)).reshape(P, GROUPS, D)
        def parr(W, f0, f1):
            # [D, F] slice -> [P, NDC, F] so each SBUF partition's data is one
            # contiguous DMA line
            w = W[:, f0:f1].reshape(NDC, P, f1 - f0)
            return np.ascontiguousarray(
                w.transpose(1, 0, 2)).astype(ml_dtypes.bfloat16)

        in_maps.append({
            "xT": np.ascontiguousarray(x[b].T).astype(ml_dtypes.bfloat16),
            "wq": parr(Wq, r * FQ, (r + 1) * FQ),
            "wk": parr(Wk, r * FKV, (r + 1) * FKV),
            "wv": parr(Wv, r * FKV, (r + 1) * FKV),
            "wo": wo_loc.astype(ml_dtypes.bfloat16),
            "cosq": cosq, "cosk": cosk, "sin": sin,
            "rqT": rqT, "rkT": rkT, "hsel": hsel, "hexp": hexp,
            "masks": masks, "ident": ident,
        })

    nc = _get_nc()
    res = run_bass_kernel_spmd(nc, in_maps, core_ids=list(range(8)), trace=trace)
    out = np.empty((B, T, D), dtype=np.float32)
    for b in range(B):
        acc = res.results[4 * b]["outT"].astype(np.float32)
        for r in range(1, 4):
            acc = acc + res.results[4 * b + r]["outT"].astype(np.float32)
        out[b] = acc.T
    return out, res


def kernel(**inputs):
    out, _ = _run(inputs, trace=False)
    return out


# revision 38
# speedup vs baseline: 1.0254x; 1.0254x over previous
"""GQA (32 q heads / 8 kv heads, head_dim 64, causal, QK-RMSNorm + RoPE) on 8 TRN2 cores.

Sharding: data-parallel over batch (2) x tensor-parallel over heads (4):
each core handles one batch element, 8 query heads, 2 kv heads, and produces
a partial output (its heads' slice of the Wo contraction); the host sums the
4 partials per batch element.

v3 (from v2 baseline):
- startup: critical-path DMA order (wv, wk, x-tb0 first; consts/trig/wq after),
  bf16 trig tables, bf16 output partials (host accumulates in f32).
- phase 2: PSUM o-accumulators evacuated to SBUF by DVE immediately after the
  AV accumulation finishes (frees the PSUM bank in ~0.6us instead of ~5us so
  the next block's AV never stalls); softmax normalize (denom Ln/Exp + GpSimd
  broadcast + DVE mult) runs from the SBUF copy and is issued LATE (interleaved
  after the next block's first exp) so the Scalar engine never delays the
  exp stream the PE is waiting on.
- causal trim: the second diagonal key-block of every (qh, g) attention block
  only computes the upper query half (256 fewer score/exp/AV columns).
- output projection drained one dc-chunk per kc step as PE filler.
"""

import numpy as np
import ml_dtypes

import concourse.bass as bass
import concourse.mybir as mybir
import concourse.tile as tile
from concourse import bacc
from concourse.bass_utils import run_bass_kernel_spmd

# Keep Ln and Exp in one activation table (natural_log_exp_and_others) so the
# ACT engine never ping-pongs table loads between them: strip exp/ln from every
# other set so the table-load pass has a single candidate for both.
import concourse.hw_specs as _hw_specs

_orig_get_tables = _hw_specs.get_activation_tables


def _patched_get_tables(arch):
    _AF = mybir.ActivationFunctionType
    tabs = dict(_orig_get_tables(arch))
    out = {}
    for name, fset in tabs.items():
        if name == "natural_log_exp_and_others":
            out[name] = set(fset)
        else:
            out[name] = set(fset) - {_AF.Exp, _AF.Ln}
    return out


_hw_specs.get_activation_tables = _patched_get_tables
bacc.get_activation_tables = _patched_get_tables


# Problem config (hardcoded per contract)
B, T, D = 2, 2048, 2048
H, KV, HD = 32, 8, 64
GROUPS = H // KV
THETA = 10000.0
SCALE = 1.0 / np.sqrt(HD)
EPS = 1e-6

# Per-core sharding
HQL = H // 4          # 8 local q heads
KVL = KV // 4         # 2 local kv heads (= groups per core)
FQ = HQL * HD         # 512
FKV = KVL * HD        # 128

# Tiling
P = 128
TB = 512              # token block (phase 1)
TQ = 256              # query sub-block (phase 2)
NTB = T // TB         # 4
NDC = D // P          # 16 contraction chunks
NKC = T // P          # 16 key chunks
NQC = FQ // P         # 4 q-proj chunks (2 heads each)

f32 = mybir.dt.float32
bf16 = mybir.dt.bfloat16
AF = mybir.ActivationFunctionType
ALU = mybir.AluOpType


def _build_nc():
    nc = bacc.Bacc("TRN2", target_bir_lowering=False, debug=False, num_devices=8)

    eps_t = nc.alloc_sbuf_tensor("const-f32-eps", [128, 1], f32)
    nc.gpsimd.memset(eps_t.ap(), EPS)
    nc.const_aps.aps[(f32, EPS)] = eps_t.ap()
    nc.all_engine_barrier()

    xT_d = nc.dram_tensor("xT", [D, T], bf16, kind="ExternalInput")
    wq_d = nc.dram_tensor("wq", [P, NDC, FQ], bf16, kind="ExternalInput")
    wk_d = nc.dram_tensor("wk", [P, NDC, FKV], bf16, kind="ExternalInput")
    wv_d = nc.dram_tensor("wv", [P, NDC, FKV], bf16, kind="ExternalInput")
    wo_d = nc.dram_tensor("wo", [P, NQC, D], bf16, kind="ExternalInput")
    cosq_d = nc.dram_tensor("cosq", [P, T], bf16, kind="ExternalInput")
    cosk_d = nc.dram_tensor("cosk", [P, T], bf16, kind="ExternalInput")
    sin_d = nc.dram_tensor("sin", [P, T], bf16, kind="ExternalInput")
    rqT_d = nc.dram_tensor("rqT", [P, P], bf16, kind="ExternalInput")
    rkT_d = nc.dram_tensor("rkT", [P, P], bf16, kind="ExternalInput")
    hsel_d = nc.dram_tensor("hsel", [P, 2], bf16, kind="ExternalInput")
    hexp_d = nc.dram_tensor("hexp", [2, P], bf16, kind="ExternalInput")
    masks_d = nc.dram_tensor("masks", [P, 2, GROUPS, TQ], bf16, kind="ExternalInput")
    ident_d = nc.dram_tensor("ident", [P, P], bf16, kind="ExternalInput")
    outT_d = nc.dram_tensor("outT", [D, T], bf16, kind="ExternalOutput")
    # DRAM bounce buffers: softmax denominators spread to 128 partitions so
    # the reciprocal ACT call is 8 columns instead of 1024
    dsc_d = nc.dram_tensor("dsc", [8, 1024], bf16, kind="Internal")
    dsc2_d = nc.dram_tensor("dsc2", [8, 1024], bf16, kind="Internal")

    with tile.TileContext(nc) as tc:
        with (
            tc.tile_pool(name="wpool", bufs=1) as wpool,
            tc.tile_pool(name="cpool", bufs=1) as cpool,
            tc.tile_pool(name="kvpool", bufs=1) as kvpool,
            tc.tile_pool(name="xpool", bufs=2) as xpool,
            tc.tile_pool(name="trig", bufs=2) as trig,
            tc.tile_pool(name="bpool", bufs=2) as bpool,
            tc.tile_pool(name="epool", bufs=6) as epool,
            tc.tile_pool(name="outp", bufs=3) as outp,
            tc.tile_pool(name="opool", bufs=3) as opool,
        ):
            # ---- persistent weights / constants ----
            wq_sb = wpool.tile([P, NDC, FQ], bf16)
            wk_sb = wpool.tile([P, NDC, FKV], bf16)
            wv_sb = wpool.tile([P, NDC, FKV], bf16)
            wo_sb = wpool.tile([P, NQC, D], bf16)
            # critical path first: V/K weights in chunks interleaved with the
            # first x chunks so the first V matmul starts as early as possible

            rqT_sb = cpool.tile([P, P], bf16)
            rkT_sb = cpool.tile([P, P], bf16)
            hsel_sb = cpool.tile([P, 2], bf16)
            hexp_sb = cpool.tile([2, P], bf16)
            masks_sb = cpool.tile([P, 2, GROUPS, TQ], bf16)
            ident_sb = cpool.tile([P, P], bf16)

            # K^T per group on partition halves; V [key, kc, g, hd+ones];
            # Q packed [64g+hd partitions, head-in-group slot, token]
            ktf = kvpool.tile([P, T], bf16)
            v_sb = kvpool.tile([P, NKC, KVL, 66], bf16)
            qg = kvpool.tile([P, GROUPS, T], bf16)
            ones_bc = nc.const_aps.tensor(1.0, (P, NKC, KVL, 66), f32)
            nc.vector.tensor_copy(v_sb[:], ones_bc)

            # ---------------- Phase 1: projections + QK norm/rope ----------
            with (
                tc.tile_pool(name="psA", bufs=6, space="PSUM") as psA,
                tc.tile_pool(name="psB", bufs=2, space="PSUM") as psB,
            ):
                def load_x(tb_l):
                    t = xpool.tile([P, NDC, TB], bf16, tag="x")
                    for dc in range(NDC):
                        nc.sync.dma_start(
                            t[:, dc, :],
                            xT_d[dc * P:(dc + 1) * P,
                                 tb_l * TB:(tb_l + 1) * TB])
                    return t

                xpre = xpool.tile([P, NDC, TB], bf16, tag="x")
                for dc in range(NDC):
                    if dc < 4:
                        nc.sync.dma_start(wv_sb[:, 4 * dc:4 * dc + 4, :],
                                          wv_d[:, 4 * dc:4 * dc + 4, :])
                    elif dc in (4, 8):
                        h = (dc - 4) // 4
                        nc.sync.dma_start(wk_sb[:, 8 * h:8 * h + 8, :],
                                          wk_d[:, 8 * h:8 * h + 8, :])
                    nc.sync.dma_start(xpre[:, dc, :],
                                      xT_d[dc * P:(dc + 1) * P, 0:TB])
                for tb in range(NTB):
                    tbs = slice(tb * TB, (tb + 1) * TB)
                    xtb = xpre

                    if tb == 0:
                        # non-critical constants behind the tb0 x chunks
                        nc.sync.dma_start(ident_sb[:], ident_d[:])
                        nc.sync.dma_start(hsel_sb[:], hsel_d[:])

                    cq_t = trig.tile([P, TB], bf16, tag="cq")
                    ck_t = trig.tile([P, TB], bf16, tag="ck")
                    sn_t = trig.tile([P, TB], bf16, tag="sn")
                    nc.sync.dma_start(cq_t[:], cosq_d[:, tbs])
                    nc.sync.dma_start(ck_t[:], cosk_d[:, tbs])
                    nc.sync.dma_start(sn_t[:], sin_d[:, tbs])

                    if tb == 0:
                        nc.sync.dma_start(rqT_sb[:], rqT_d[:])
                        nc.sync.dma_start(rkT_sb[:], rkT_d[:])
                        nc.sync.dma_start(hexp_sb[:], hexp_d[:])
                        # wq quarters interleaved with the tb1 x prefetch so
                        # neither the tb0 Q projection nor the tb1 V/K
                        # projection waits on a monolithic transfer
                        xpre = xpool.tile([P, NDC, TB], bf16, tag="x")
                        for q4 in range(4):
                            nc.sync.dma_start(wq_sb[:, 4 * q4:4 * q4 + 4, :],
                                              wq_d[:, 4 * q4:4 * q4 + 4, :])
                            for dc in range(4 * q4, 4 * q4 + 4):
                                nc.sync.dma_start(
                                    xpre[:, dc, :],
                                    xT_d[dc * P:(dc + 1) * P, TB:2 * TB])
                        nc.sync.dma_start(masks_sb[:], masks_d[:])
                    elif tb == 1:
                        nc.sync.dma_start(wo_sb[:], wo_d[:])
                        xpre = load_x(2)
                    elif tb == 2:
                        xpre = load_x(3)

                    # projections: V, K, then 4 Q chunks
                    vps = psA.tile([P, TB], f32, tag="big", name="vps")
                    for dc in range(NDC):
                        nc.tensor.matmul(vps[:], wv_sb[:, dc, :], xtb[:, dc, :],
                                         start=dc == 0, stop=dc == NDC - 1)
                    vt = bpool.tile([P, TB], bf16, tag="vt")
                    nc.scalar.copy(vt[:], vps[:])
                    # V transpose on the PE (bf16), both groups per 128-chunk
                    for st4 in range(TB // P):
                        kc = tb * (TB // P) + st4
                        tp = psB.tile([P, P], bf16, tag="small", name="tp")
                        nc.tensor.transpose(tp[:], vt[:, st4 * P:(st4 + 1) * P],
                                            ident_sb[:])
                        nc.vector.tensor_copy(v_sb[:, kc, 0, 0:64], tp[:, 0:64])
                        nc.vector.tensor_copy(v_sb[:, kc, 1, 0:64], tp[:, 64:P])

                    kps = psA.tile([P, TB], f32, tag="big", name="kps")
                    for dc in range(NDC):
                        nc.tensor.matmul(kps[:], wk_sb[:, dc, :], xtb[:, dc, :],
                                         start=dc == 0, stop=dc == NDC - 1)

                    qps = [psA.tile([P, TB], f32, tag="big", name=f"qps{c}")
                           for c in range(NQC)]
                    for dc in range(NDC):
                        for c in range(NQC):
                            nc.tensor.matmul(qps[c][:],
                                             wq_sb[:, dc, c * P:(c + 1) * P],
                                             xtb[:, dc, :],
                                             start=dc == 0, stop=dc == NDC - 1)

                    # pass 1: bf16 copies + per-token sum of squares
                    chunks = qps + [kps]
                    qsb = []
                    ss_sb = bpool.tile([2, NQC + 1, TB], f32, tag="ss_sb", bufs=1)
                    for ci, cps in enumerate(chunks):
                        qs_t = bpool.tile([P, TB], bf16, tag=f"qsb{ci}")
                        nc.scalar.copy(qs_t[:], cps[:])
                        qsb.append(qs_t)
                        sq = bpool.tile([P, TB], bf16, tag="sq")
                        nc.vector.tensor_tensor(sq[:], qs_t[:], qs_t[:], ALU.mult)
                        ssp = psB.tile([2, TB], f32, tag="small", name="ssp")
                        nc.tensor.matmul(ssp[:], hsel_sb[:], sq[:],
                                         start=True, stop=True)
                        nc.scalar.copy(ss_sb[:, ci, :], ssp[:])

                    # per-chunk rsqrt: rr = exp(-0.5 * ln(ss/HD + eps));
                    # split per ci so pass 2 of ci=0 starts without waiting
                    # the full batch through the ACT queue
                    rr = bpool.tile([2, NQC + 1, TB], bf16, tag="rr", bufs=1)
                    for ci in range(NQC + 1):
                        lnb = bpool.tile([2, TB], f32, tag="lnb")
                        nc.scalar.activation(lnb[:], ss_sb[:, ci, :], AF.Ln,
                                             bias=EPS, scale=1.0 / HD)
                        nc.scalar.activation(rr[:, ci, :], lnb[:], AF.Exp,
                                             scale=-0.5)

                    # pass 2: rope + apply rsqrt, write qg / ktf
                    for ci in range(NQC + 1):
                        is_k = ci == NQC
                        rT = rkT_sb if is_k else rqT_sb
                        ct = ck_t if is_k else cq_t
                        bc = psB.tile([P, TB], f32, tag="small", name="bc")
                        nc.tensor.matmul(bc[:], hexp_sb[:], rr[:, ci, :],
                                         start=True, stop=True)
                        rot = psB.tile([P, TB], f32, tag="small", name="rot")
                        nc.tensor.matmul(rot[:], rT[:], qsb[ci][:],
                                         start=True, stop=True)
                        m1 = bpool.tile([P, TB], bf16, tag="m1")
                        nc.vector.tensor_tensor(m1[:], qsb[ci][:], ct[:], ALU.mult)
                        m2 = bpool.tile([P, TB], bf16, tag="m2")
                        nc.vector.tensor_tensor(m2[:], rot[:], sn_t[:], ALU.mult)
                        s12 = bpool.tile([P, TB], bf16, tag="s12")
                        nc.vector.tensor_tensor(s12[:], m1[:], m2[:], ALU.add)
                        if not is_k:
                            g = ci // 2
                            j0 = 2 * (ci % 2)
                            gp = slice(64 * g, 64 * g + 64)
                            nc.vector.tensor_tensor(qg[gp, j0, tbs],
                                                    s12[0:64], bc[0:64], ALU.mult)
                            nc.vector.tensor_tensor(qg[gp, j0 + 1, tbs],
                                                    s12[64:P], bc[64:P], ALU.mult)
                        else:
                            nc.vector.tensor_tensor(ktf[0:64, tbs],
                                                    s12[0:64], bc[0:64], ALU.mult)
                            nc.vector.tensor_tensor(ktf[64:P, tbs],
                                                    s12[64:P], bc[64:P], ALU.mult)

            # ---------------- Phase 2: attention + output projection --------
            with (
                tc.tile_pool(name="psSP", bufs=2, space="PSUM") as psSP,
                tc.tile_pool(name="psO", bufs=2, space="PSUM") as psO,
                tc.tile_pool(name="psM", bufs=2, space="PSUM") as psM,
            ):
                pending = []       # queued output-projection thunks (PE filler)
                norm2 = []         # deferred normalize mults (DVE)
                nblk = [0]         # block counter for DRAM bounce slots

                def queue_E(tb_e, orhs_e, dc2_range, half=None, spread=False):
                    ts0 = tb_e * TB if half is None else tb_e * TB + half * TQ
                    w = TB if half is None else TQ
                    cs = slice(0, TB) if half is None else slice(half * TQ,
                                                                 (half + 1) * TQ)
                    for dc2 in dc2_range:
                        def th(dc2=dc2):
                            acc = psM.tile([P, w], f32, tag="m", name="acc")
                            for j in range(GROUPS):
                                nc.tensor.matmul(acc[:],
                                                 wo_sb[:, j, dc2 * P:(dc2 + 1) * P],
                                                 orhs_e[:, j, cs],
                                                 start=j == 0, stop=j == GROUPS - 1)
                            ob = outp.tile([P, w], bf16, tag="ob")
                            nc.vector.tensor_copy(ob[:], acc[:])
                            nc.sync.dma_start(
                                outT_d[dc2 * P:(dc2 + 1) * P, ts0:ts0 + w], ob[:])
                        pending.append(th)

                def drain(n=1):
                    for _ in range(n):
                        if pending:
                            pending.pop(0)()

                for tb in range(NTB):
                    orhs = bpool.tile([P, GROUPS, TB], bf16, tag="orhs")
                    last = tb == NTB - 1
                    blocks = [(qh, g) for qh in range(2) for g in range(KVL)]
                    for bi, (qh, g) in enumerate(blocks):
                        gp = slice(64 * g, 64 * g + 64)
                        qbase = tb * TB + qh * TQ
                        qs = slice(qbase, qbase + TQ)
                        qsl = slice(qh * TQ, (qh + 1) * TQ)
                        nkc = qbase // P + 2
                        o01 = psO.tile([65, 2, TQ], f32, tag="o", name="o01")
                        o23 = psO.tile([65, 2, TQ], f32, tag="o", name="o23")
                        es_l = [None] * nkc
                        trim_l = [False] * nkc
                        # software pipeline: AV(kc) trails exp(kc) by one step
                        for kc in range(nkc + 1):
                            if kc < nkc:
                                trim = kc == nkc - 1  # 2nd diagonal block:
                                # queries 0..127 of this TQ fully masked
                                trim_l[kc] = trim
                                if trim:
                                    qv = slice(qbase + TQ // 2, qbase + TQ)
                                    hh = slice(0, TQ // 2)
                                    sps = psSP.tile([P, GROUPS, TQ], f32,
                                                    tag="sps")
                                    nc.tensor.matmul(
                                        sps[:, 0:2, hh],
                                        ktf[gp, kc * P:(kc + 1) * P],
                                        qg[gp, 0:2, qv], start=True, stop=True)
                                    nc.tensor.matmul(
                                        sps[:, 2:4, hh],
                                        ktf[gp, kc * P:(kc + 1) * P],
                                        qg[gp, 2:4, qv], start=True, stop=True)
                                    es = epool.tile([P, GROUPS, TQ], bf16,
                                                    tag="es")
                                    nc.scalar.activation(es[:, :, hh],
                                                         sps[:, :, hh], AF.Exp,
                                                         scale=float(SCALE))
                                    nc.vector.tensor_tensor(
                                        es[:, :, hh], es[:, :, hh],
                                        masks_sb[:, 0, :, 0:TQ // 2], ALU.mult)
                                else:
                                    sps = psSP.tile([P, GROUPS, TQ], f32,
                                                    tag="sps")
                                    nc.tensor.matmul(
                                        sps[:, 0:2, :],
                                        ktf[gp, kc * P:(kc + 1) * P],
                                        qg[gp, 0:2, qs], start=True, stop=True)
                                    nc.tensor.matmul(
                                        sps[:, 2:4, :],
                                        ktf[gp, kc * P:(kc + 1) * P],
                                        qg[gp, 2:4, qs], start=True, stop=True)
                                    es = epool.tile([P, GROUPS, TQ], bf16,
                                                    tag="es")
                                    nc.scalar.activation(es[:], sps[:], AF.Exp,
                                                         scale=float(SCALE))
                                    if kc == nkc - 2:
                                        # 1st diagonal block: triangle mask
                                        nc.vector.tensor_tensor(
                                            es[:], es[:],
                                            masks_sb[:, 0, :, :], ALU.mult)
                                es_l[kc] = es
                            # outproj filler between score(kc) and AV(kc-1)
                            # keeps the PE busy while exp(kc-1) finishes
                            drain()
                            if kc == 2 and nkc >= 6:
                                # long block: flush deferred normalize mults
                                # here (denominator chain has settled, and the
                                # DVE is idle until this block's mask mults)
                                while norm2:
                                    norm2.pop(0)()
                            if kc >= 1:
                                kp = kc - 1
                                st = kp == 0
                                sp = kp == nkc - 1
                                if trim_l[kp]:
                                    hq = slice(TQ // 2, TQ)
                                    hh = slice(0, TQ // 2)
                                    nc.tensor.matmul(
                                        o01[:, :, hq], v_sb[:, kp, g, 0:65],
                                        es_l[kp][:, 0:2, hh], start=st, stop=sp)
                                    nc.tensor.matmul(
                                        o23[:, :, hq], v_sb[:, kp, g, 0:65],
                                        es_l[kp][:, 2:4, hh], start=st, stop=sp)
                                else:
                                    nc.tensor.matmul(
                                        o01[:], v_sb[:, kp, g, 0:65],
                                        es_l[kp][:, 0:2, :], start=st, stop=sp)
                                    nc.tensor.matmul(
                                        o23[:], v_sb[:, kp, g, 0:65],
                                        es_l[kp][:, 2:4, :], start=st, stop=sp)
                        # evacuate PSUM accumulators to SBUF immediately (DVE)
                        # so the next block's AV reuses the banks without
                        # waiting for the normalize chain
                        o_sb = opool.tile([65, 2, 2, TQ], bf16, tag="osb")
                        nc.vector.tensor_copy(o_sb[:, 0, :, :], o01[:])
                        nc.vector.tensor_copy(o_sb[:, 1, :, :], o23[:])
                        while norm2:
                            norm2.pop(0)()

                        final = last and bi == len(blocks) - 1
                        if final:
                            # tail latency matters: recip straight off the
                            # denom row, broadcast on the (now idle) PE
                            dln = bpool.tile([1, 2, 2, TQ], f32, tag="dln")
                            nc.scalar.activation(dln[:], o_sb[64:65, :, :, :],
                                                 AF.Ln)
                            den = bpool.tile([1, 2, 2, TQ], bf16, tag="den")
                            nc.scalar.activation(den[:], dln[:], AF.Exp,
                                                 scale=-1.0)
                            for h in range(2):
                                bch = psM.tile([64, 2, TQ], f32, tag="m",
                                               name="bch")
                                nc.tensor.matmul(bch[:], hexp_sb[0:1, 0:64],
                                                 den[0:1, h, :, :],
                                                 start=True, stop=True)
                                nc.vector.tensor_tensor(
                                    orhs[gp, 2 * h:2 * h + 2, qsl],
                                    o_sb[0:64, h, :, :], bch[:], ALU.mult)
                        else:
                            # denom recip: bounce through DRAM to spread the
                            # 1024 values over 128 partitions so the ACT
                            # passes cost 8 columns instead of 1024
                            slot = nblk[0] % 8
                            nblk[0] += 1
                            nc.sync.dma_start(dsc_d[slot:slot + 1, :],
                                              o_sb[64:65, :, :, :])
                            dsp = bpool.tile([P, 8], bf16, tag="dsp")
                            nc.sync.dma_start(
                                dsp[:],
                                dsc_d[slot:slot + 1, :].rearrange(
                                    "o (p c) -> (o p) c", p=P, c=8))
                            dl8 = bpool.tile([P, 8], f32, tag="dl8")
                            nc.scalar.activation(dl8[:], dsp[:], AF.Ln)
                            dr8 = bpool.tile([P, 8], bf16, tag="dr8")
                            nc.scalar.activation(dr8[:], dl8[:], AF.Exp,
                                                 scale=-1.0)
                            nc.sync.dma_start(
                                dsc2_d[slot:slot + 1, :].rearrange(
                                    "o (p c) -> (o p) c", p=P, c=8), dr8[:])
                            drow = bpool.tile([1, 2, 2, TQ], bf16, tag="drow")
                            nc.sync.dma_start(drow[:], dsc2_d[slot:slot + 1, :])
                            bc2 = bpool.tile([64, 2, 2, TQ], bf16, tag="bc2")
                            nc.gpsimd.partition_broadcast(bc2[:], drow[:])

                            # the normalize mults go on DVE but DEFERRED one
                            # block, so they sit behind the next block's
                            # evacuation in the DVE queue and never delay it
                            # while the denominator DMA chain is in flight
                            def n2(o_sb=o_sb, bc2=bc2, gp=gp, qsl=qsl,
                                   orhs=orhs):
                                nc.vector.tensor_tensor(
                                    orhs[gp, 0:2, qsl], o_sb[0:64, 0, :, :],
                                    bc2[:, 0, :, :], ALU.mult)
                                nc.vector.tensor_tensor(
                                    orhs[gp, 2:4, qsl], o_sb[0:64, 1, :, :],
                                    bc2[:, 1, :, :], ALU.mult)
                            norm2.append(n2)

                        # last tb only: queue the first-half output projection
                        # before the final block (there is no later work to
                        # drain it against); all other tbs queue both halves
                        # at tb end so a drain never pops before its orhs is
                        # normalized
                        if bi == 2 and last:
                            queue_E(tb, orhs, range(NDC), half=0)
                    while norm2:
                        norm2.pop(0)()
                    if not last:
                        queue_E(tb, orhs, range(NDC), half=0)
                    queue_E(tb, orhs, range(NDC), half=1)
                drain(len(pending))
                drain(len(pending))

    nc.compile()
    return nc


_NC_CACHE = None


def _get_nc():
    global _NC_CACHE
    if _NC_CACHE is None:
        _NC_CACHE = _build_nc()
    return _NC_CACHE


def _host_constants(q_scale, k_scale):
    pos = np.arange(T, dtype=np.float64)
    invf = 1.0 / (THETA ** (np.arange(0, HD, 2, dtype=np.float64) / HD))  # (32,)
    ang = pos[:, None] * invf[None, :]                                    # (T, 32)
    c = np.cos(ang)
    s = np.sin(ang)
    pidx = np.arange(P) % 32
    hidx = np.arange(P) % HD
    cosq = (c[:, pidx].T * q_scale[hidx][:, None]).astype(ml_dtypes.bfloat16)
    cosk = (c[:, pidx].T * k_scale[hidx][:, None]).astype(ml_dtypes.bfloat16)
    sin = s[:, pidx].T.astype(ml_dtypes.bfloat16)

    def rmat(scale):
        R = np.zeros((HD, HD), dtype=np.float64)
        for i in range(32):
            R[i, i + 32] = -scale[i + 32]
            R[i + 32, i] = scale[i]
        M = np.kron(np.eye(2), R)
        return np.ascontiguousarray(M.T).astype(ml_dtypes.bfloat16)

    hsel = np.zeros((P, 2), dtype=np.float32)
    hsel[0:64, 0] = 1.0
    hsel[64:P, 1] = 1.0
    hexp = np.ascontiguousarray(hsel.T).astype(ml_dtypes.bfloat16)
    hsel = hsel.astype(ml_dtypes.bfloat16)

    # masks[p, i, j, f] = (f >= p + 128*i), replicated over the 4 head slots
    pp = np.arange(P)[:, None]
    ff = np.arange(TQ)[None, :]
    masks = np.zeros((P, 2, GROUPS, TQ), dtype=np.float32)
    for i in range(2):
        m = (ff >= pp + P * i).astype(np.float32)
        for j in range(GROUPS):
            masks[:, i, j, :] = m
    masks = masks.astype(ml_dtypes.bfloat16)
    ident = np.eye(P, dtype=ml_dtypes.bfloat16)

    return cosq, cosk, sin, rmat(q_scale), rmat(k_scale), hsel, hexp, masks, ident


def _run(inputs, trace=False):
    x = np.asarray(inputs["x"], dtype=np.float32)
    Wq = np.asarray(inputs["Wq"], dtype=np.float32)
    Wk = np.asarray(inputs["Wk"], dtype=np.float32)
    Wv = np.asarray(inputs["Wv"], dtype=np.float32)
    Wo = np.asarray(inputs["Wo"], dtype=np.float32)
    q_scale = np.asarray(inputs["q_scale"], dtype=np.float64)
    k_scale = np.asarray(inputs["k_scale"], dtype=np.float64)

    cosq, cosk, sin, rqT, rkT, hsel, hexp, masks, ident = _host_constants(
        q_scale, k_scale)

    in_maps = []
    for cid in range(8):
        b = cid // 4
        r = cid % 4
        # Wo rows (g, head j within group, hd) -> [64g+hd partitions, j slots]
        wo_loc = Wo[r * FQ:(r + 1) * FQ, :].reshape(KVL, GROUPS, HD, D)
        wo_loc = np.ascontiguousarray(
            wo_loc.transpose(0, 2, 1, 3)).reshape(P, GROUPS, D)
        def parr(W, f0, f1):
            # [D, F] slice -> [P, NDC, F] so each SBUF partition's data is one
            # contiguous DMA line
            w = W[:, f0:f1].reshape(NDC, P, f1 - f0)
            return np.ascontiguousarray(
                w.transpose(1, 0, 2)).astype(ml_dtypes.bfloat16)

        in_maps.append({
            "xT": np.ascontiguousarray(x[b].T).astype(ml_dtypes.bfloat16),
            "wq": parr(Wq, r * FQ, (r + 1) * FQ),
            "wk": parr(Wk, r * FKV, (r + 1) * FKV),
            "wv": parr(Wv, r * FKV, (r + 1) * FKV),
            "wo": wo_loc.astype(ml_dtypes.bfloat16),
            "cosq": cosq, "cosk": cosk, "sin": sin,
            "rqT": rqT, "rkT": rkT, "hsel": hsel, "hexp": hexp,
            "masks": masks, "ident": ident,
        })

    nc = _get_nc()
    res = run_bass_kernel_spmd(nc, in_maps, core_ids=list(range(8)), trace=trace)
    out = np.empty((B, T, D), dtype=np.float32)
    for b in range(B):
        acc = res.results[4 * b]["outT"].astype(np.float32)
        for r in range(1, 4):
            acc = acc + res.results[4 * b + r]["outT"].astype(np.float32)
        out[b] = acc.T
    return out, res


def kernel(**inputs):
    out, _ = _run(inputs, trace=False)
    return out


# revision 40
# speedup vs baseline: 1.0326x; 1.0070x over previous
"""GQA (32 q heads / 8 kv heads, head_dim 64, causal, QK-RMSNorm + RoPE) on 8 TRN2 cores.

Sharding: data-parallel over batch (2) x tensor-parallel over heads (4):
each core handles one batch element, 8 query heads, 2 kv heads, and produces
a partial output (its heads' slice of the Wo contraction); the host sums the
4 partials per batch element.

v3 (from v2 baseline):
- startup: critical-path DMA order (wv, wk, x-tb0 first; consts/trig/wq after),
  bf16 trig tables, bf16 output partials (host accumulates in f32).
- phase 2: PSUM o-accumulators evacuated to SBUF by DVE immediately after the
  AV accumulation finishes (frees the PSUM bank in ~0.6us instead of ~5us so
  the next block's AV never stalls); softmax normalize (denom Ln/Exp + GpSimd
  broadcast + DVE mult) runs from the SBUF copy and is issued LATE (interleaved
  after the next block's first exp) so the Scalar engine never delays the
  exp stream the PE is waiting on.
- causal trim: the second diagonal key-block of every (qh, g) attention block
  only computes the upper query half (256 fewer score/exp/AV columns).
- output projection drained one dc-chunk per kc step as PE filler.
"""

import numpy as np
import ml_dtypes

import concourse.bass as bass
import concourse.mybir as mybir
import concourse.tile as tile
from concourse import bacc
from concourse.bass_utils import run_bass_kernel_spmd

# Keep Ln and Exp in one activation table (natural_log_exp_and_others) so the
# ACT engine never ping-pongs table loads between them: strip exp/ln from every
# other set so the table-load pass has a single candidate for both.
import concourse.hw_specs as _hw_specs

_orig_get_tables = _hw_specs.get_activation_tables


def _patched_get_tables(arch):
    _AF = mybir.ActivationFunctionType
    tabs = dict(_orig_get_tables(arch))
    out = {}
    for name, fset in tabs.items():
        if name == "natural_log_exp_and_others":
            out[name] = set(fset)
        else:
            out[name] = set(fset) - {_AF.Exp, _AF.Ln}
    return out


_hw_specs.get_activation_tables = _patched_get_tables
bacc.get_activation_tables = _patched_get_tables


# Problem config (hardcoded per contract)
B, T, D = 2, 2048, 2048
H, KV, HD = 32, 8, 64
GROUPS = H // KV
THETA = 10000.0
SCALE = 1.0 / np.sqrt(HD)
EPS = 1e-6

# Per-core sharding
HQL = H // 4          # 8 local q heads
KVL = KV // 4         # 2 local kv heads (= groups per core)
FQ = HQL * HD         # 512
FKV = KVL * HD        # 128

# Tiling
P = 128
TB = 512              # token block (phase 1)
TQ = 256              # query sub-block (phase 2)
NTB = T // TB         # 4
NDC = D // P          # 16 contraction chunks
NKC = T // P          # 16 key chunks
NQC = FQ // P         # 4 q-proj chunks (2 heads each)

f32 = mybir.dt.float32
bf16 = mybir.dt.bfloat16
AF = mybir.ActivationFunctionType
ALU = mybir.AluOpType


def _build_nc():
    nc = bacc.Bacc("TRN2", target_bir_lowering=False, debug=False, num_devices=8)

    eps_t = nc.alloc_sbuf_tensor("const-f32-eps", [128, 1], f32)
    nc.gpsimd.memset(eps_t.ap(), EPS)
    nc.const_aps.aps[(f32, EPS)] = eps_t.ap()
    nc.all_engine_barrier()

    xT_d = nc.dram_tensor("xT", [D, T], bf16, kind="ExternalInput")
    wq_d = nc.dram_tensor("wq", [P, NDC, FQ], bf16, kind="ExternalInput")
    wk_d = nc.dram_tensor("wk", [P, NDC, FKV], bf16, kind="ExternalInput")
    wv_d = nc.dram_tensor("wv", [P, NDC, FKV], bf16, kind="ExternalInput")
    wo_d = nc.dram_tensor("wo", [P, NQC, D], bf16, kind="ExternalInput")
    cosq_d = nc.dram_tensor("cosq", [P, T], bf16, kind="ExternalInput")
    cosk_d = nc.dram_tensor("cosk", [P, T], bf16, kind="ExternalInput")
    sin_d = nc.dram_tensor("sin", [P, T], bf16, kind="ExternalInput")
    rqT_d = nc.dram_tensor("rqT", [P, P], bf16, kind="ExternalInput")
    rkT_d = nc.dram_tensor("rkT", [P, P], bf16, kind="ExternalInput")
    hsel_d = nc.dram_tensor("hsel", [P, 2], bf16, kind="ExternalInput")
    hexp_d = nc.dram_tensor("hexp", [2, P], bf16, kind="ExternalInput")
    masks_d = nc.dram_tensor("masks", [P, 2, GROUPS, TQ], bf16, kind="ExternalInput")
    ident_d = nc.dram_tensor("ident", [P, P], bf16, kind="ExternalInput")
    outT_d = nc.dram_tensor("outT", [D, T], bf16, kind="ExternalOutput")
    # final query-half output in packed per-partition layout: 8KB DMA lines
    # instead of 512B, so the closing DMA costs ~128 descriptors, not 2048
    outF_d = nc.dram_tensor("outF", [P, NDC, TQ], bf16, kind="ExternalOutput")
    # DRAM bounce buffers: softmax denominators spread to 128 partitions so
    # the reciprocal ACT call is 8 columns instead of 1024
    dsc_d = nc.dram_tensor("dsc", [8, 1024], bf16, kind="Internal")
    dsc2_d = nc.dram_tensor("dsc2", [8, 1024], bf16, kind="Internal")

    with tile.TileContext(nc) as tc:
        with (
            tc.tile_pool(name="wpool", bufs=1) as wpool,
            tc.tile_pool(name="cpool", bufs=1) as cpool,
            tc.tile_pool(name="kvpool", bufs=1) as kvpool,
            tc.tile_pool(name="xpool", bufs=2) as xpool,
            tc.tile_pool(name="trig", bufs=2) as trig,
            tc.tile_pool(name="bpool", bufs=2) as bpool,
            tc.tile_pool(name="epool", bufs=6) as epool,
            tc.tile_pool(name="outp", bufs=3) as outp,
            tc.tile_pool(name="opool", bufs=3) as opool,
        ):
            # ---- persistent weights / constants ----
            wq_sb = wpool.tile([P, NDC, FQ], bf16)
            wk_sb = wpool.tile([P, NDC, FKV], bf16)
            wv_sb = wpool.tile([P, NDC, FKV], bf16)
            wo_sb = wpool.tile([P, NQC, D], bf16)
            # critical path first: V/K weights in chunks interleaved with the
            # first x chunks so the first V matmul starts as early as possible

            rqT_sb = cpool.tile([P, P], bf16)
            rkT_sb = cpool.tile([P, P], bf16)
            hsel_sb = cpool.tile([P, 2], bf16)
            hexp_sb = cpool.tile([2, P], bf16)
            masks_sb = cpool.tile([P, 2, GROUPS, TQ], bf16)
            ident_sb = cpool.tile([P, P], bf16)

            # K^T per group on partition halves; V [key, kc, g, hd+ones];
            # Q packed [64g+hd partitions, head-in-group slot, token]
            ktf = kvpool.tile([P, T], bf16)
            v_sb = kvpool.tile([P, NKC, KVL, 66], bf16)
            qg = kvpool.tile([P, GROUPS, T], bf16)
            ones_bc = nc.const_aps.tensor(1.0, (P, NKC, KVL, 66), f32)
            nc.vector.tensor_copy(v_sb[:], ones_bc)

            # ---------------- Phase 1: projections + QK norm/rope ----------
            with (
                tc.tile_pool(name="psA", bufs=6, space="PSUM") as psA,
                tc.tile_pool(name="psB", bufs=2, space="PSUM") as psB,
            ):
                def load_x(tb_l):
                    t = xpool.tile([P, NDC, TB], bf16, tag="x")
                    for dc in range(NDC):
                        nc.sync.dma_start(
                            t[:, dc, :],
                            xT_d[dc * P:(dc + 1) * P,
                                 tb_l * TB:(tb_l + 1) * TB])
                    return t

                xpre = xpool.tile([P, NDC, TB], bf16, tag="x")
                for dc in range(NDC):
                    if dc < 4:
                        nc.sync.dma_start(wv_sb[:, 4 * dc:4 * dc + 4, :],
                                          wv_d[:, 4 * dc:4 * dc + 4, :])
                    elif dc in (4, 8):
                        h = (dc - 4) // 4
                        nc.sync.dma_start(wk_sb[:, 8 * h:8 * h + 8, :],
                                          wk_d[:, 8 * h:8 * h + 8, :])
                    nc.sync.dma_start(xpre[:, dc, :],
                                      xT_d[dc * P:(dc + 1) * P, 0:TB])
                for tb in range(NTB):
                    tbs = slice(tb * TB, (tb + 1) * TB)
                    xtb = xpre

                    if tb == 0:
                        # non-critical constants behind the tb0 x chunks
                        nc.sync.dma_start(ident_sb[:], ident_d[:])
                        nc.sync.dma_start(hsel_sb[:], hsel_d[:])

                    cq_t = trig.tile([P, TB], bf16, tag="cq")
                    ck_t = trig.tile([P, TB], bf16, tag="ck")
                    sn_t = trig.tile([P, TB], bf16, tag="sn")
                    nc.sync.dma_start(cq_t[:], cosq_d[:, tbs])
                    nc.sync.dma_start(ck_t[:], cosk_d[:, tbs])
                    nc.sync.dma_start(sn_t[:], sin_d[:, tbs])

                    if tb == 0:
                        nc.sync.dma_start(rqT_sb[:], rqT_d[:])
                        nc.sync.dma_start(rkT_sb[:], rkT_d[:])
                        nc.sync.dma_start(hexp_sb[:], hexp_d[:])
                        # wq quarters interleaved with the tb1 x prefetch so
                        # neither the tb0 Q projection nor the tb1 V/K
                        # projection waits on a monolithic transfer
                        xpre = xpool.tile([P, NDC, TB], bf16, tag="x")
                        for q4 in range(4):
                            nc.sync.dma_start(wq_sb[:, 4 * q4:4 * q4 + 4, :],
                                              wq_d[:, 4 * q4:4 * q4 + 4, :])
                            for dc in range(4 * q4, 4 * q4 + 4):
                                nc.sync.dma_start(
                                    xpre[:, dc, :],
                                    xT_d[dc * P:(dc + 1) * P, TB:2 * TB])
                        nc.sync.dma_start(masks_sb[:], masks_d[:])
                    elif tb == 1:
                        nc.sync.dma_start(wo_sb[:], wo_d[:])
                        xpre = load_x(2)
                    elif tb == 2:
                        xpre = load_x(3)

                    # projections: V, K, then 4 Q chunks
                    vps = psA.tile([P, TB], f32, tag="big", name="vps")
                    for dc in range(NDC):
                        nc.tensor.matmul(vps[:], wv_sb[:, dc, :], xtb[:, dc, :],
                                         start=dc == 0, stop=dc == NDC - 1)
                    vt = bpool.tile([P, TB], bf16, tag="vt")
                    nc.scalar.copy(vt[:], vps[:])
                    # V transpose on the PE (bf16), both groups per 128-chunk
                    for st4 in range(TB // P):
                        kc = tb * (TB // P) + st4
                        tp = psB.tile([P, P], bf16, tag="small", name="tp")
                        nc.tensor.transpose(tp[:], vt[:, st4 * P:(st4 + 1) * P],
                                            ident_sb[:])
                        nc.vector.tensor_copy(v_sb[:, kc, 0, 0:64], tp[:, 0:64])
                        nc.vector.tensor_copy(v_sb[:, kc, 1, 0:64], tp[:, 64:P])

                    kps = psA.tile([P, TB], f32, tag="big", name="kps")
                    for dc in range(NDC):
                        nc.tensor.matmul(kps[:], wk_sb[:, dc, :], xtb[:, dc, :],
                                         start=dc == 0, stop=dc == NDC - 1)

                    qps = [psA.tile([P, TB], f32, tag="big", name=f"qps{c}")
                           for c in range(NQC)]
                    for dc in range(NDC):
                        for c in range(NQC):
                            nc.tensor.matmul(qps[c][:],
                                             wq_sb[:, dc, c * P:(c + 1) * P],
                                             xtb[:, dc, :],
                                             start=dc == 0, stop=dc == NDC - 1)

                    # pass 1: bf16 copies + per-token sum of squares
                    chunks = qps + [kps]
                    qsb = []
                    ss_sb = bpool.tile([2, NQC + 1, TB], f32, tag="ss_sb", bufs=1)
                    for ci, cps in enumerate(chunks):
                        qs_t = bpool.tile([P, TB], bf16, tag=f"qsb{ci}")
                        nc.scalar.copy(qs_t[:], cps[:])
                        qsb.append(qs_t)
                        sq = bpool.tile([P, TB], bf16, tag="sq")
                        nc.vector.tensor_tensor(sq[:], qs_t[:], qs_t[:], ALU.mult)
                        ssp = psB.tile([2, TB], f32, tag="small", name="ssp")
                        nc.tensor.matmul(ssp[:], hsel_sb[:], sq[:],
                                         start=True, stop=True)
                        nc.scalar.copy(ss_sb[:, ci, :], ssp[:])

                    # per-chunk rsqrt: rr = exp(-0.5 * ln(ss/HD + eps));
                    # split per ci so pass 2 of ci=0 starts without waiting
                    # the full batch through the ACT queue
                    rr = bpool.tile([2, NQC + 1, TB], bf16, tag="rr", bufs=1)
                    for ci in range(NQC + 1):
                        lnb = bpool.tile([2, TB], f32, tag="lnb")
                        nc.scalar.activation(lnb[:], ss_sb[:, ci, :], AF.Ln,
                                             bias=EPS, scale=1.0 / HD)
                        nc.scalar.activation(rr[:, ci, :], lnb[:], AF.Exp,
                                             scale=-0.5)

                    # pass 2: rope + apply rsqrt, write qg / ktf
                    for ci in range(NQC + 1):
                        is_k = ci == NQC
                        rT = rkT_sb if is_k else rqT_sb
                        ct = ck_t if is_k else cq_t
                        bc = psB.tile([P, TB], f32, tag="small", name="bc")
                        nc.tensor.matmul(bc[:], hexp_sb[:], rr[:, ci, :],
                                         start=True, stop=True)
                        rot = psB.tile([P, TB], f32, tag="small", name="rot")
                        nc.tensor.matmul(rot[:], rT[:], qsb[ci][:],
                                         start=True, stop=True)
                        m1 = bpool.tile([P, TB], bf16, tag="m1")
                        nc.vector.tensor_tensor(m1[:], qsb[ci][:], ct[:], ALU.mult)
                        m2 = bpool.tile([P, TB], bf16, tag="m2")
                        nc.vector.tensor_tensor(m2[:], rot[:], sn_t[:], ALU.mult)
                        s12 = bpool.tile([P, TB], bf16, tag="s12")
                        nc.vector.tensor_tensor(s12[:], m1[:], m2[:], ALU.add)
                        if not is_k:
                            g = ci // 2
                            j0 = 2 * (ci % 2)
                            gp = slice(64 * g, 64 * g + 64)
                            nc.vector.tensor_tensor(qg[gp, j0, tbs],
                                                    s12[0:64], bc[0:64], ALU.mult)
                            nc.vector.tensor_tensor(qg[gp, j0 + 1, tbs],
                                                    s12[64:P], bc[64:P], ALU.mult)
                        else:
                            nc.vector.tensor_tensor(ktf[0:64, tbs],
                                                    s12[0:64], bc[0:64], ALU.mult)
                            nc.vector.tensor_tensor(ktf[64:P, tbs],
                                                    s12[64:P], bc[64:P], ALU.mult)

            # ---------------- Phase 2: attention + output projection --------
            with (
                tc.tile_pool(name="psSP", bufs=2, space="PSUM") as psSP,
                tc.tile_pool(name="psO", bufs=2, space="PSUM") as psO,
                tc.tile_pool(name="psM", bufs=2, space="PSUM") as psM,
            ):
                pending = []       # queued output-projection thunks (PE filler)
                norm2 = []         # deferred normalize mults (DVE)
                nblk = [0]         # block counter for DRAM bounce slots

                def queue_E(tb_e, orhs_e, dc2_range, half=None, fstage=None):
                    ts0 = tb_e * TB if half is None else tb_e * TB + half * TQ
                    w = TB if half is None else TQ
                    cs = slice(0, TB) if half is None else slice(half * TQ,
                                                                 (half + 1) * TQ)
                    for dc2 in dc2_range:
                        def th(dc2=dc2):
                            acc = psM.tile([P, w], f32, tag="m", name="acc")
                            for j in range(GROUPS):
                                nc.tensor.matmul(acc[:],
                                                 wo_sb[:, j, dc2 * P:(dc2 + 1) * P],
                                                 orhs_e[:, j, cs],
                                                 start=j == 0, stop=j == GROUPS - 1)
                            if fstage is not None:
                                # packed epilogue: big-line DMA every 4 chunks
                                nc.vector.tensor_copy(fstage[:, dc2, :], acc[:])
                                if dc2 % 4 == 3:
                                    nc.sync.dma_start(
                                        outF_d[:, dc2 - 3:dc2 + 1, :],
                                        fstage[:, dc2 - 3:dc2 + 1, :])
                                return
                            ob = outp.tile([P, w], bf16, tag="ob")
                            nc.vector.tensor_copy(ob[:], acc[:])
                            nc.sync.dma_start(
                                outT_d[dc2 * P:(dc2 + 1) * P, ts0:ts0 + w], ob[:])
                        pending.append(th)

                def drain(n=1):
                    for _ in range(n):
                        if pending:
                            pending.pop(0)()

                for tb in range(NTB):
                    orhs = bpool.tile([P, GROUPS, TB], bf16, tag="orhs")
                    last = tb == NTB - 1
                    blocks = [(qh, g) for qh in range(2) for g in range(KVL)]
                    for bi, (qh, g) in enumerate(blocks):
                        gp = slice(64 * g, 64 * g + 64)
                        qbase = tb * TB + qh * TQ
                        qs = slice(qbase, qbase + TQ)
                        qsl = slice(qh * TQ, (qh + 1) * TQ)
                        nkc = qbase // P + 2
                        o01 = psO.tile([65, 2, TQ], f32, tag="o", name="o01")
                        o23 = psO.tile([65, 2, TQ], f32, tag="o", name="o23")
                        es_l = [None] * nkc
                        trim_l = [False] * nkc
                        # software pipeline: AV(kc) trails exp(kc) by one step
                        for kc in range(nkc + 1):
                            if kc < nkc:
                                trim = kc == nkc - 1  # 2nd diagonal block:
                                # queries 0..127 of this TQ fully masked
                                trim_l[kc] = trim
                                if trim:
                                    qv = slice(qbase + TQ // 2, qbase + TQ)
                                    hh = slice(0, TQ // 2)
                                    sps = psSP.tile([P, GROUPS, TQ], f32,
                                                    tag="sps")
                                    nc.tensor.matmul(
                                        sps[:, 0:2, hh],
                                        ktf[gp, kc * P:(kc + 1) * P],
                                        qg[gp, 0:2, qv], start=True, stop=True)
                                    nc.tensor.matmul(
                                        sps[:, 2:4, hh],
                                        ktf[gp, kc * P:(kc + 1) * P],
                                        qg[gp, 2:4, qv], start=True, stop=True)
                                    es = epool.tile([P, GROUPS, TQ], bf16,
                                                    tag="es")
                                    nc.scalar.activation(es[:, :, hh],
                                                         sps[:, :, hh], AF.Exp,
                                                         scale=float(SCALE))
                                    nc.vector.tensor_tensor(
                                        es[:, :, hh], es[:, :, hh],
                                        masks_sb[:, 0, :, 0:TQ // 2], ALU.mult)
                                else:
                                    sps = psSP.tile([P, GROUPS, TQ], f32,
                                                    tag="sps")
                                    nc.tensor.matmul(
                                        sps[:, 0:2, :],
                                        ktf[gp, kc * P:(kc + 1) * P],
                                        qg[gp, 0:2, qs], start=True, stop=True)
                                    nc.tensor.matmul(
                                        sps[:, 2:4, :],
                                        ktf[gp, kc * P:(kc + 1) * P],
                                        qg[gp, 2:4, qs], start=True, stop=True)
                                    es = epool.tile([P, GROUPS, TQ], bf16,
                                                    tag="es")
                                    nc.scalar.activation(es[:], sps[:], AF.Exp,
                                                         scale=float(SCALE))
                                    if kc == nkc - 2:
                                        # 1st diagonal block: triangle mask
                                        nc.vector.tensor_tensor(
                                            es[:], es[:],
                                            masks_sb[:, 0, :, :], ALU.mult)
                                es_l[kc] = es
                            # outproj filler between score(kc) and AV(kc-1)
                            # keeps the PE busy while exp(kc-1) finishes
                            drain()
                            if kc == 2 and nkc >= 6:
                                # long block: flush deferred normalize mults
                                # here (denominator chain has settled, and the
                                # DVE is idle until this block's mask mults)
                                while norm2:
                                    norm2.pop(0)()
                            if kc >= 1:
                                kp = kc - 1
                                st = kp == 0
                                sp = kp == nkc - 1
                                if trim_l[kp]:
                                    hq = slice(TQ // 2, TQ)
                                    hh = slice(0, TQ // 2)
                                    nc.tensor.matmul(
                                        o01[:, :, hq], v_sb[:, kp, g, 0:65],
                                        es_l[kp][:, 0:2, hh], start=st, stop=sp)
                                    nc.tensor.matmul(
                                        o23[:, :, hq], v_sb[:, kp, g, 0:65],
                                        es_l[kp][:, 2:4, hh], start=st, stop=sp)
                                else:
                                    nc.tensor.matmul(
                                        o01[:], v_sb[:, kp, g, 0:65],
                                        es_l[kp][:, 0:2, :], start=st, stop=sp)
                                    nc.tensor.matmul(
                                        o23[:], v_sb[:, kp, g, 0:65],
                                        es_l[kp][:, 2:4, :], start=st, stop=sp)
                        # evacuate PSUM accumulators to SBUF immediately (DVE)
                        # so the next block's AV reuses the banks without
                        # waiting for the normalize chain
                        o_sb = opool.tile([65, 2, 2, TQ], bf16, tag="osb")
                        nc.vector.tensor_copy(o_sb[:, 0, :, :], o01[:])
                        nc.vector.tensor_copy(o_sb[:, 1, :, :], o23[:])
                        while norm2:
                            norm2.pop(0)()

                        final = last and bi == len(blocks) - 1
                        if final:
                            # tail latency matters: recip straight off the
                            # denom row, broadcast on the (now idle) PE
                            dln = bpool.tile([1, 2, 2, TQ], f32, tag="dln")
                            nc.scalar.activation(dln[:], o_sb[64:65, :, :, :],
                                                 AF.Ln)
                            den = bpool.tile([1, 2, 2, TQ], bf16, tag="den")
                            nc.scalar.activation(den[:], dln[:], AF.Exp,
                                                 scale=-1.0)
                            for h in range(2):
                                bch = psM.tile([64, 2, TQ], f32, tag="m",
                                               name="bch")
                                nc.tensor.matmul(bch[:], hexp_sb[0:1, 0:64],
                                                 den[0:1, h, :, :],
                                                 start=True, stop=True)
                                nc.vector.tensor_tensor(
                                    orhs[gp, 2 * h:2 * h + 2, qsl],
                                    o_sb[0:64, h, :, :], bch[:], ALU.mult)
                        else:
                            # denom recip: bounce through DRAM to spread the
                            # 1024 values over 128 partitions so the ACT
                            # passes cost 8 columns instead of 1024
                            slot = nblk[0] % 8
                            nblk[0] += 1
                            nc.sync.dma_start(dsc_d[slot:slot + 1, :],
                                              o_sb[64:65, :, :, :])
                            dsp = bpool.tile([P, 8], bf16, tag="dsp")
                            nc.sync.dma_start(
                                dsp[:],
                                dsc_d[slot:slot + 1, :].rearrange(
                                    "o (p c) -> (o p) c", p=P, c=8))
                            dl8 = bpool.tile([P, 8], f32, tag="dl8")
                            nc.scalar.activation(dl8[:], dsp[:], AF.Ln)
                            dr8 = bpool.tile([P, 8], bf16, tag="dr8")
                            nc.scalar.activation(dr8[:], dl8[:], AF.Exp,
                                                 scale=-1.0)
                            nc.sync.dma_start(
                                dsc2_d[slot:slot + 1, :].rearrange(
                                    "o (p c) -> (o p) c", p=P, c=8), dr8[:])
                            drow = bpool.tile([1, 2, 2, TQ], bf16, tag="drow")
                            nc.sync.dma_start(drow[:], dsc2_d[slot:slot + 1, :])
                            bc2 = bpool.tile([64, 2, 2, TQ], bf16, tag="bc2")
                            nc.gpsimd.partition_broadcast(bc2[:], drow[:])

                            # the normalize mults go on DVE but DEFERRED one
                            # block, so they sit behind the next block's
                            # evacuation in the DVE queue and never delay it
                            # while the denominator DMA chain is in flight
                            def n2(o_sb=o_sb, bc2=bc2, gp=gp, qsl=qsl,
                                   orhs=orhs):
                                nc.vector.tensor_tensor(
                                    orhs[gp, 0:2, qsl], o_sb[0:64, 0, :, :],
                                    bc2[:, 0, :, :], ALU.mult)
                                nc.vector.tensor_tensor(
                                    orhs[gp, 2:4, qsl], o_sb[0:64, 1, :, :],
                                    bc2[:, 1, :, :], ALU.mult)
                            norm2.append(n2)

                        # last tb only: queue the first-half output projection
                        # before the final block (there is no later work to
                        # drain it against); all other tbs queue both halves
                        # at tb end so a drain never pops before its orhs is
                        # normalized
                        if bi == 2 and last:
                            queue_E(tb, orhs, range(NDC), half=0)
                    while norm2:
                        norm2.pop(0)()
                    if not last:
                        queue_E(tb, orhs, range(NDC), half=0)
                        queue_E(tb, orhs, range(NDC), half=1)
                    else:
                        fst = kvpool.tile([P, NDC, TQ], bf16)
                        queue_E(tb, orhs, range(NDC), half=1, fstage=fst)
                drain(len(pending))
                drain(len(pending))

    nc.compile()
    return nc


_NC_CACHE = None


def _get_nc():
    global _NC_CACHE
    if _NC_CACHE is None:
        _NC_CACHE = _build_nc()
    return _NC_CACHE


def _host_constants(q_scale, k_scale):
    pos = np.arange(T, dtype=np.float64)
    invf = 1.0 / (THETA ** (np.arange(0, HD, 2, dtype=np.float64) / HD))  # (32,)
    ang = pos[:, None] * invf[None, :]                                    # (T, 32)
    c = np.cos(ang)
    s = np.sin(ang)
    pidx = np.arange(P) % 32
    hidx = np.arange(P) % HD
    cosq = (c[:, pidx].T * q_scale[hidx][:, None]).astype(ml_dtypes.bfloat16)
    cosk = (c[:, pidx].T * k_scale[hidx][:, None]).astype(ml_dtypes.bfloat16)
    sin = s[:, pidx].T.astype(ml_dtypes.bfloat16)

    def rmat(scale):
        R = np.zeros((HD, HD), dtype=np.float64)
        for i in range(32):
            R[i, i + 32] = -scale[i + 32]
            R[i + 32, i] = scale[i]
        M = np.kron(np.eye(2), R)
        return np.ascontiguousarray(M.T).astype(ml_dtypes.bfloat16)

    hsel = np.zeros((P, 2), dtype=np.float32)
    hsel[0:64, 0] = 1.0
    hsel[64:P, 1] = 1.0
    hexp = np.ascontiguousarray(hsel.T).astype(ml_dtypes.bfloat16)
    hsel = hsel.astype(ml_dtypes.bfloat16)

    # masks[p, i, j, f] = (f >= p + 128*i), replicated over the 4 head slots
    pp = np.arange(P)[:, None]
    ff = np.arange(TQ)[None, :]
    masks = np.zeros((P, 2, GROUPS, TQ), dtype=np.float32)
    for i in range(2):
        m = (ff >= pp + P * i).astype(np.float32)
        for j in range(GROUPS):
            masks[:, i, j, :] = m
    masks = masks.astype(ml_dtypes.bfloat16)
    ident = np.eye(P, dtype=ml_dtypes.bfloat16)

    return cosq, cosk, sin, rmat(q_scale), rmat(k_scale), hsel, hexp, masks, ident


def _run(inputs, trace=False):
    x = np.asarray(inputs["x"], dtype=np.float32)
    Wq = np.asarray(inputs["Wq"], dtype=np.float32)
    Wk = np.asarray(inputs["Wk"], dtype=np.float32)
    Wv = np.asarray(inputs["Wv"], dtype=np.float32)
    Wo = np.asarray(inputs["Wo"], dtype=np.float32)
    q_scale = np.asarray(inputs["q_scale"], dtype=np.float64)
    k_scale = np.asarray(inputs["k_scale"], dtype=np.float64)

    cosq, cosk, sin, rqT, rkT, hsel, hexp, masks, ident = _host_constants(
        q_scale, k_scale)

    in_maps = []
    for cid in range(8):
        b = cid // 4
        r = cid % 4
        # Wo rows (g, head j within group, hd) -> [64g+hd partitions, j slots]
        wo_loc = Wo[r * FQ:(r + 1) * FQ, :].reshape(KVL, GROUPS, HD, D)
        wo_loc = np.ascontiguousarray(
            wo_loc.transpose(0, 2, 1, 3)).reshape(P, GROUPS, D)
        def parr(W, f0, f1):
            # [D, F] slice -> [P, NDC, F] so each SBUF partition's data is one
            # contiguous DMA line
            w = W[:, f0:f1].reshape(NDC, P, f1 - f0)
            return np.ascontiguousarray(
                w.transpose(1, 0, 2)).astype(ml_dtypes.bfloat16)

        in_maps.append({
            "xT": np.ascontiguousarray(x[b].T).astype(ml_dtypes.bfloat16),
            "wq": parr(Wq, r * FQ, (r + 1) * FQ),
            "wk": parr(Wk, r * FKV, (r + 1) * FKV),
            "wv": parr(Wv, r * FKV, (r + 1) * FKV),
            "wo": wo_loc.astype(ml_dtypes.bfloat16),
            "cosq": cosq, "cosk": cosk, "sin": sin,
            "rqT": rqT, "rkT": rkT, "hsel": hsel, "hexp": hexp,
            "masks": masks, "ident": ident,
        })

    nc = _get_nc()
    res = run_bass_kernel_spmd(nc, in_maps, core_ids=list(range(8)), trace=trace)
    out = np.empty((B, T, D), dtype=np.float32)
    for b in range(B):
        acc = res.results[4 * b]["outT"].astype(np.float32)
        accF = res.results[4 * b]["outF"].astype(np.float32)
        for r in range(1, 4):
            acc = acc + res.results[4 * b + r]["outT"].astype(np.float32)
            accF = accF + res.results[4 * b + r]["outF"].astype(np.float32)
        # outF is the packed final query-half: [P, NDC, TQ] -> [D, TQ]
        acc[:, T - TQ:T] = accF.transpose(1, 0, 2).reshape(D, TQ)
        out[b] = acc.T
    return out, res


def kernel(**inputs):
    out, _ = _run(inputs, trace=False)
    return out


# revision 41
# speedup vs baseline: 1.0356x; 1.0029x over previous
"""GQA (32 q heads / 8 kv heads, head_dim 64, causal, QK-RMSNorm + RoPE) on 8 TRN2 cores.

Sharding: data-parallel over batch (2) x tensor-parallel over heads (4):
each core handles one batch element, 8 query heads, 2 kv heads, and produces
a partial output (its heads' slice of the Wo contraction); the host sums the
4 partials per batch element.

v4 (from v2 baseline, 474us -> ~385us):
- startup: critical-path DMA order (wv/wk chunks interleaved with tb0 x, then
  wq quarters interleaved with the tb1 x prefetch), host-prearranged weight
  layouts for contiguous DMA lines, bf16 trig tables, bf16 output partials
  (host accumulates in f32).
- phase 2: PSUM o-accumulators evacuated to SBUF by DVE immediately after the
  AV accumulation stops (frees the bank in <1us so the next block's AV never
  waits); softmax denominator reciprocal bounced through DRAM to spread the
  1024 values over 128 partitions (ACT recip costs 8 columns, not 1024, so
  the Scalar engine stays dedicated to the exp stream the PE waits on);
  normalize mults deferred one block behind the evacuations on DVE.
- causal trim: the second diagonal key-block of every (qh, g) attention block
  only computes the upper query half (256 fewer score/exp/AV columns).
- output projection drained one dc-chunk per kc step, placed between the
  score and AV matmuls as PE filler covering the exp latency; the final
  query-half is written via a packed scratch output (8KB DMA lines, host
  unpacks) to cut the closing descriptor-generation tail.
"""

import numpy as np
import ml_dtypes

import concourse.bass as bass
import concourse.mybir as mybir
import concourse.tile as tile
from concourse import bacc
from concourse.bass_utils import run_bass_kernel_spmd

# Keep Ln and Exp in one activation table (natural_log_exp_and_others) so the
# ACT engine never ping-pongs table loads between them: strip exp/ln from every
# other set so the table-load pass has a single candidate for both.
import concourse.hw_specs as _hw_specs

_orig_get_tables = _hw_specs.get_activation_tables


def _patched_get_tables(arch):
    _AF = mybir.ActivationFunctionType
    tabs = dict(_orig_get_tables(arch))
    out = {}
    for name, fset in tabs.items():
        if name == "natural_log_exp_and_others":
            out[name] = set(fset)
        else:
            out[name] = set(fset) - {_AF.Exp, _AF.Ln}
    return out


_hw_specs.get_activation_tables = _patched_get_tables
bacc.get_activation_tables = _patched_get_tables


# Problem config (hardcoded per contract)
B, T, D = 2, 2048, 2048
H, KV, HD = 32, 8, 64
GROUPS = H // KV
THETA = 10000.0
SCALE = 1.0 / np.sqrt(HD)
EPS = 1e-6

# Per-core sharding
HQL = H // 4          # 8 local q heads
KVL = KV // 4         # 2 local kv heads (= groups per core)
FQ = HQL * HD         # 512
FKV = KVL * HD        # 128

# Tiling
P = 128
TB = 512              # token block (phase 1)
TQ = 256              # query sub-block (phase 2)
NTB = T // TB         # 4
NDC = D // P          # 16 contraction chunks
NKC = T // P          # 16 key chunks
NQC = FQ // P         # 4 q-proj chunks (2 heads each)

f32 = mybir.dt.float32
bf16 = mybir.dt.bfloat16
AF = mybir.ActivationFunctionType
ALU = mybir.AluOpType


def _build_nc():
    nc = bacc.Bacc("TRN2", target_bir_lowering=False, debug=False, num_devices=8)

    eps_t = nc.alloc_sbuf_tensor("const-f32-eps", [128, 1], f32)
    nc.gpsimd.memset(eps_t.ap(), EPS)
    nc.const_aps.aps[(f32, EPS)] = eps_t.ap()
    nc.all_engine_barrier()

    xT_d = nc.dram_tensor("xT", [D, T], bf16, kind="ExternalInput")
    wq_d = nc.dram_tensor("wq", [P, NDC, FQ], bf16, kind="ExternalInput")
    wk_d = nc.dram_tensor("wk", [P, NDC, FKV], bf16, kind="ExternalInput")
    wv_d = nc.dram_tensor("wv", [P, NDC, FKV], bf16, kind="ExternalInput")
    wo_d = nc.dram_tensor("wo", [P, NQC, D], bf16, kind="ExternalInput")
    cosq_d = nc.dram_tensor("cosq", [P, T], bf16, kind="ExternalInput")
    cosk_d = nc.dram_tensor("cosk", [P, T], bf16, kind="ExternalInput")
    sin_d = nc.dram_tensor("sin", [P, T], bf16, kind="ExternalInput")
    rqT_d = nc.dram_tensor("rqT", [P, P], bf16, kind="ExternalInput")
    rkT_d = nc.dram_tensor("rkT", [P, P], bf16, kind="ExternalInput")
    hsel_d = nc.dram_tensor("hsel", [P, 2], bf16, kind="ExternalInput")
    hexp_d = nc.dram_tensor("hexp", [2, P], bf16, kind="ExternalInput")
    masks_d = nc.dram_tensor("masks", [P, 2, GROUPS, TQ], bf16, kind="ExternalInput")
    ident_d = nc.dram_tensor("ident", [P, P], bf16, kind="ExternalInput")
    outT_d = nc.dram_tensor("outT", [D, T], bf16, kind="ExternalOutput")
    # final query-half output in packed per-partition layout: 8KB DMA lines
    # instead of 512B, so the closing DMA costs ~128 descriptors, not 2048
    outF_d = nc.dram_tensor("outF", [P, NDC, TQ], bf16, kind="ExternalOutput")
    # DRAM bounce buffers: softmax denominators spread to 128 partitions so
    # the reciprocal ACT call is 8 columns instead of 1024
    dsc_d = nc.dram_tensor("dsc", [8, 1024], bf16, kind="Internal")
    dsc2_d = nc.dram_tensor("dsc2", [8, 1024], bf16, kind="Internal")

    with tile.TileContext(nc) as tc:
        with (
            tc.tile_pool(name="wpool", bufs=1) as wpool,
            tc.tile_pool(name="cpool", bufs=1) as cpool,
            tc.tile_pool(name="kvpool", bufs=1) as kvpool,
            tc.tile_pool(name="xpool", bufs=2) as xpool,
            tc.tile_pool(name="trig", bufs=2) as trig,
            tc.tile_pool(name="bpool", bufs=2) as bpool,
            tc.tile_pool(name="epool", bufs=6) as epool,
            tc.tile_pool(name="outp", bufs=3) as outp,
            tc.tile_pool(name="opool", bufs=3) as opool,
        ):
            # ---- persistent weights / constants ----
            wq_sb = wpool.tile([P, NDC, FQ], bf16)
            wk_sb = wpool.tile([P, NDC, FKV], bf16)
            wv_sb = wpool.tile([P, NDC, FKV], bf16)
            wo_sb = wpool.tile([P, NQC, D], bf16)
            # critical path first: V/K weights in chunks interleaved with the
            # first x chunks so the first V matmul starts as early as possible

            rqT_sb = cpool.tile([P, P], bf16)
            rkT_sb = cpool.tile([P, P], bf16)
            hsel_sb = cpool.tile([P, 2], bf16)
            hexp_sb = cpool.tile([2, P], bf16)
            masks_sb = cpool.tile([P, 2, GROUPS, TQ], bf16)
            ident_sb = cpool.tile([P, P], bf16)

            # K^T per group on partition halves; V [key, kc, g, hd+ones];
            # Q packed [64g+hd partitions, head-in-group slot, token]
            ktf = kvpool.tile([P, T], bf16)
            v_sb = kvpool.tile([P, NKC, KVL, 66], bf16)
            qg = kvpool.tile([P, GROUPS, T], bf16)
            ones_bc = nc.const_aps.tensor(1.0, (P, NKC, KVL, 66), f32)
            nc.vector.tensor_copy(v_sb[:], ones_bc)

            # ---------------- Phase 1: projections + QK norm/rope ----------
            with (
                tc.tile_pool(name="psA", bufs=6, space="PSUM") as psA,
                tc.tile_pool(name="psB", bufs=2, space="PSUM") as psB,
            ):
                def load_x(tb_l):
                    t = xpool.tile([P, NDC, TB], bf16, tag="x")
                    for dc in range(NDC):
                        nc.sync.dma_start(
                            t[:, dc, :],
                            xT_d[dc * P:(dc + 1) * P,
                                 tb_l * TB:(tb_l + 1) * TB])
                    return t

                xpre = xpool.tile([P, NDC, TB], bf16, tag="x")
                for dc in range(NDC):
                    if dc < 4:
                        nc.sync.dma_start(wv_sb[:, 4 * dc:4 * dc + 4, :],
                                          wv_d[:, 4 * dc:4 * dc + 4, :])
                    elif dc in (4, 8):
                        h = (dc - 4) // 4
                        nc.sync.dma_start(wk_sb[:, 8 * h:8 * h + 8, :],
                                          wk_d[:, 8 * h:8 * h + 8, :])
                    nc.sync.dma_start(xpre[:, dc, :],
                                      xT_d[dc * P:(dc + 1) * P, 0:TB])
                for tb in range(NTB):
                    tbs = slice(tb * TB, (tb + 1) * TB)
                    xtb = xpre

                    if tb == 0:
                        # non-critical constants behind the tb0 x chunks
                        nc.sync.dma_start(ident_sb[:], ident_d[:])
                        nc.sync.dma_start(hsel_sb[:], hsel_d[:])

                    cq_t = trig.tile([P, TB], bf16, tag="cq")
                    ck_t = trig.tile([P, TB], bf16, tag="ck")
                    sn_t = trig.tile([P, TB], bf16, tag="sn")
                    nc.sync.dma_start(cq_t[:], cosq_d[:, tbs])
                    nc.sync.dma_start(ck_t[:], cosk_d[:, tbs])
                    nc.sync.dma_start(sn_t[:], sin_d[:, tbs])

                    if tb == 0:
                        nc.sync.dma_start(rqT_sb[:], rqT_d[:])
                        nc.sync.dma_start(rkT_sb[:], rkT_d[:])
                        nc.sync.dma_start(hexp_sb[:], hexp_d[:])
                        # wq quarters interleaved with the tb1 x prefetch so
                        # neither the tb0 Q projection nor the tb1 V/K
                        # projection waits on a monolithic transfer
                        xpre = xpool.tile([P, NDC, TB], bf16, tag="x")
                        for q4 in range(4):
                            nc.sync.dma_start(wq_sb[:, 4 * q4:4 * q4 + 4, :],
                                              wq_d[:, 4 * q4:4 * q4 + 4, :])
                            for dc in range(4 * q4, 4 * q4 + 4):
                                nc.sync.dma_start(
                                    xpre[:, dc, :],
                                    xT_d[dc * P:(dc + 1) * P, TB:2 * TB])
                        nc.sync.dma_start(masks_sb[:], masks_d[:])
                    elif tb == 1:
                        nc.sync.dma_start(wo_sb[:], wo_d[:])
                        xpre = load_x(2)
                    elif tb == 2:
                        xpre = load_x(3)

                    # projections: V, K, then 4 Q chunks
                    vps = psA.tile([P, TB], f32, tag="big", name="vps")
                    for dc in range(NDC):
                        nc.tensor.matmul(vps[:], wv_sb[:, dc, :], xtb[:, dc, :],
                                         start=dc == 0, stop=dc == NDC - 1)
                    vt = bpool.tile([P, TB], bf16, tag="vt")
                    nc.scalar.copy(vt[:], vps[:])
                    # V transpose on the PE (bf16), both groups per 128-chunk
                    for st4 in range(TB // P):
                        kc = tb * (TB // P) + st4
                        tp = psB.tile([P, P], bf16, tag="small", name="tp")
                        nc.tensor.transpose(tp[:], vt[:, st4 * P:(st4 + 1) * P],
                                            ident_sb[:])
                        nc.vector.tensor_copy(v_sb[:, kc, 0, 0:64], tp[:, 0:64])
                        nc.vector.tensor_copy(v_sb[:, kc, 1, 0:64], tp[:, 64:P])

                    kps = psA.tile([P, TB], f32, tag="big", name="kps")
                    for dc in range(NDC):
                        nc.tensor.matmul(kps[:], wk_sb[:, dc, :], xtb[:, dc, :],
                                         start=dc == 0, stop=dc == NDC - 1)

                    qps = [psA.tile([P, TB], f32, tag="big", name=f"qps{c}")
                           for c in range(NQC)]
                    for dc in range(NDC):
                        for c in range(NQC):
                            nc.tensor.matmul(qps[c][:],
                                             wq_sb[:, dc, c * P:(c + 1) * P],
                                             xtb[:, dc, :],
                                             start=dc == 0, stop=dc == NDC - 1)

                    # pass 1: bf16 copies + per-token sum of squares
                    chunks = qps + [kps]
                    qsb = []
                    ss_sb = bpool.tile([2, NQC + 1, TB], f32, tag="ss_sb", bufs=1)
                    for ci, cps in enumerate(chunks):
                        qs_t = bpool.tile([P, TB], bf16, tag=f"qsb{ci}")
                        nc.scalar.copy(qs_t[:], cps[:])
                        qsb.append(qs_t)
                        sq = bpool.tile([P, TB], bf16, tag="sq")
                        nc.vector.tensor_tensor(sq[:], qs_t[:], qs_t[:], ALU.mult)
                        ssp = psB.tile([2, TB], f32, tag="small", name="ssp")
                        nc.tensor.matmul(ssp[:], hsel_sb[:], sq[:],
                                         start=True, stop=True)
                        nc.scalar.copy(ss_sb[:, ci, :], ssp[:])

                    # per-chunk rsqrt: rr = exp(-0.5 * ln(ss/HD + eps));
                    # split per ci so pass 2 of ci=0 starts without waiting
                    # the full batch through the ACT queue
                    rr = bpool.tile([2, NQC + 1, TB], bf16, tag="rr", bufs=1)
                    for ci in range(NQC + 1):
                        lnb = bpool.tile([2, TB], f32, tag="lnb")
                        nc.scalar.activation(lnb[:], ss_sb[:, ci, :], AF.Ln,
                                             bias=EPS, scale=1.0 / HD)
                        nc.scalar.activation(rr[:, ci, :], lnb[:], AF.Exp,
                                             scale=-0.5)

                    # pass 2: rope + apply rsqrt, write qg / ktf
                    for ci in range(NQC + 1):
                        is_k = ci == NQC
                        rT = rkT_sb if is_k else rqT_sb
                        ct = ck_t if is_k else cq_t
                        bc = psB.tile([P, TB], f32, tag="small", name="bc")
                        nc.tensor.matmul(bc[:], hexp_sb[:], rr[:, ci, :],
                                         start=True, stop=True)
                        rot = psB.tile([P, TB], f32, tag="small", name="rot")
                        nc.tensor.matmul(rot[:], rT[:], qsb[ci][:],
                                         start=True, stop=True)
                        m1 = bpool.tile([P, TB], bf16, tag="m1")
                        nc.vector.tensor_tensor(m1[:], qsb[ci][:], ct[:], ALU.mult)
                        m2 = bpool.tile([P, TB], bf16, tag="m2")
                        nc.vector.tensor_tensor(m2[:], rot[:], sn_t[:], ALU.mult)
                        s12 = bpool.tile([P, TB], bf16, tag="s12")
                        nc.vector.tensor_tensor(s12[:], m1[:], m2[:], ALU.add)
                        if not is_k:
                            g = ci // 2
                            j0 = 2 * (ci % 2)
                            gp = slice(64 * g, 64 * g + 64)
                            nc.vector.tensor_tensor(qg[gp, j0, tbs],
                                                    s12[0:64], bc[0:64], ALU.mult)
                            nc.vector.tensor_tensor(qg[gp, j0 + 1, tbs],
                                                    s12[64:P], bc[64:P], ALU.mult)
                        else:
                            nc.vector.tensor_tensor(ktf[0:64, tbs],
                                                    s12[0:64], bc[0:64], ALU.mult)
                            nc.vector.tensor_tensor(ktf[64:P, tbs],
                                                    s12[64:P], bc[64:P], ALU.mult)

            # ---------------- Phase 2: attention + output projection --------
            with (
                tc.tile_pool(name="psSP", bufs=2, space="PSUM") as psSP,
                tc.tile_pool(name="psO", bufs=2, space="PSUM") as psO,
                tc.tile_pool(name="psM", bufs=2, space="PSUM") as psM,
            ):
                pending = []       # queued output-projection thunks (PE filler)
                norm2 = []         # deferred normalize mults (DVE)
                nblk = [0]         # block counter for DRAM bounce slots

                def queue_E(tb_e, orhs_e, dc2_range, half=None, fstage=None):
                    ts0 = tb_e * TB if half is None else tb_e * TB + half * TQ
                    w = TB if half is None else TQ
                    cs = slice(0, TB) if half is None else slice(half * TQ,
                                                                 (half + 1) * TQ)
                    for dc2 in dc2_range:
                        def th(dc2=dc2):
                            acc = psM.tile([P, w], f32, tag="m", name="acc")
                            for j in range(GROUPS):
                                nc.tensor.matmul(acc[:],
                                                 wo_sb[:, j, dc2 * P:(dc2 + 1) * P],
                                                 orhs_e[:, j, cs],
                                                 start=j == 0, stop=j == GROUPS - 1)
                            if fstage is not None:
                                # packed epilogue: big-line DMA every 4 chunks
                                nc.vector.tensor_copy(fstage[:, dc2, :], acc[:])
                                if dc2 % 4 == 3:
                                    nc.sync.dma_start(
                                        outF_d[:, dc2 - 3:dc2 + 1, :],
                                        fstage[:, dc2 - 3:dc2 + 1, :])
                                return
                            ob = outp.tile([P, w], bf16, tag="ob")
                            nc.vector.tensor_copy(ob[:], acc[:])
                            nc.sync.dma_start(
                                outT_d[dc2 * P:(dc2 + 1) * P, ts0:ts0 + w], ob[:])
                        pending.append(th)

                def drain(n=1):
                    for _ in range(n):
                        if pending:
                            pending.pop(0)()

                for tb in range(NTB):
                    orhs = bpool.tile([P, GROUPS, TB], bf16, tag="orhs")
                    last = tb == NTB - 1
                    blocks = [(qh, g) for qh in range(2) for g in range(KVL)]
                    for bi, (qh, g) in enumerate(blocks):
                        gp = slice(64 * g, 64 * g + 64)
                        qbase = tb * TB + qh * TQ
                        qs = slice(qbase, qbase + TQ)
                        qsl = slice(qh * TQ, (qh + 1) * TQ)
                        nkc = qbase // P + 2
                        o01 = psO.tile([65, 2, TQ], f32, tag="o", name="o01")
                        o23 = psO.tile([65, 2, TQ], f32, tag="o", name="o23")
                        es_l = [None] * nkc
                        trim_l = [False] * nkc
                        # software pipeline: AV(kc) trails exp(kc) by one step
                        for kc in range(nkc + 1):
                            if kc < nkc:
                                trim = kc == nkc - 1  # 2nd diagonal block:
                                # queries 0..127 of this TQ fully masked
                                trim_l[kc] = trim
                                if trim:
                                    qv = slice(qbase + TQ // 2, qbase + TQ)
                                    hh = slice(0, TQ // 2)
                                    sps = psSP.tile([P, GROUPS, TQ], f32,
                                                    tag="sps")
                                    nc.tensor.matmul(
                                        sps[:, 0:2, hh],
                                        ktf[gp, kc * P:(kc + 1) * P],
                                        qg[gp, 0:2, qv], start=True, stop=True)
                                    nc.tensor.matmul(
                                        sps[:, 2:4, hh],
                                        ktf[gp, kc * P:(kc + 1) * P],
                                        qg[gp, 2:4, qv], start=True, stop=True)
                                    es = epool.tile([P, GROUPS, TQ], bf16,
                                                    tag="es")
                                    nc.scalar.activation(es[:, :, hh],
                                                         sps[:, :, hh], AF.Exp,
                                                         scale=float(SCALE))
                                    nc.vector.tensor_tensor(
                                        es[:, :, hh], es[:, :, hh],
                                        masks_sb[:, 0, :, 0:TQ // 2], ALU.mult)
                                else:
                                    sps = psSP.tile([P, GROUPS, TQ], f32,
                                                    tag="sps")
                                    nc.tensor.matmul(
                                        sps[:, 0:2, :],
                                        ktf[gp, kc * P:(kc + 1) * P],
                                        qg[gp, 0:2, qs], start=True, stop=True)
                                    nc.tensor.matmul(
                                        sps[:, 2:4, :],
                                        ktf[gp, kc * P:(kc + 1) * P],
                                        qg[gp, 2:4, qs], start=True, stop=True)
                                    es = epool.tile([P, GROUPS, TQ], bf16,
                                                    tag="es")
                                    nc.scalar.activation(es[:], sps[:], AF.Exp,
                                                         scale=float(SCALE))
                                    if kc == nkc - 2:
                                        # 1st diagonal block: triangle mask
                                        nc.vector.tensor_tensor(
                                            es[:], es[:],
                                            masks_sb[:, 0, :, :], ALU.mult)
                                es_l[kc] = es
                            # outproj filler between score(kc) and AV(kc-1)
                            # keeps the PE busy while exp(kc-1) finishes
                            drain()
                            if kc == 2 and nkc >= 6:
                                # long block: flush deferred normalize mults
                                # here (denominator chain has settled, and the
                                # DVE is idle until this block's mask mults)
                                while norm2:
                                    norm2.pop(0)()
                            if kc >= 1:
                                kp = kc - 1
                                st = kp == 0
                                sp = kp == nkc - 1
                                if trim_l[kp]:
                                    hq = slice(TQ // 2, TQ)
                                    hh = slice(0, TQ // 2)
                                    nc.tensor.matmul(
                                        o01[:, :, hq], v_sb[:, kp, g, 0:65],
                                        es_l[kp][:, 0:2, hh], start=st, stop=sp)
                                    nc.tensor.matmul(
                                        o23[:, :, hq], v_sb[:, kp, g, 0:65],
                                        es_l[kp][:, 2:4, hh], start=st, stop=sp)
                                else:
                                    nc.tensor.matmul(
                                        o01[:], v_sb[:, kp, g, 0:65],
                                        es_l[kp][:, 0:2, :], start=st, stop=sp)
                                    nc.tensor.matmul(
                                        o23[:], v_sb[:, kp, g, 0:65],
                                        es_l[kp][:, 2:4, :], start=st, stop=sp)
                        # evacuate PSUM accumulators to SBUF immediately (DVE)
                        # so the next block's AV reuses the banks without
                        # waiting for the normalize chain
                        o_sb = opool.tile([65, 2, 2, TQ], bf16, tag="osb")
                        nc.vector.tensor_copy(o_sb[:, 0, :, :], o01[:])
                        nc.vector.tensor_copy(o_sb[:, 1, :, :], o23[:])
                        while norm2:
                            norm2.pop(0)()

                        final = last and bi == len(blocks) - 1
                        if final:
                            # tail latency matters: recip straight off the
                            # denom row, broadcast on the (now idle) PE
                            dln = bpool.tile([1, 2, 2, TQ], f32, tag="dln")
                            nc.scalar.activation(dln[:], o_sb[64:65, :, :, :],
                                                 AF.Ln)
                            den = bpool.tile([1, 2, 2, TQ], bf16, tag="den")
                            nc.scalar.activation(den[:], dln[:], AF.Exp,
                                                 scale=-1.0)
                            for h in range(2):
                                bch = psM.tile([64, 2, TQ], f32, tag="m",
                                               name="bch")
                                nc.tensor.matmul(bch[:], hexp_sb[0:1, 0:64],
                                                 den[0:1, h, :, :],
                                                 start=True, stop=True)
                                nc.vector.tensor_tensor(
                                    orhs[gp, 2 * h:2 * h + 2, qsl],
                                    o_sb[0:64, h, :, :], bch[:], ALU.mult)
                        else:
                            # denom recip: bounce through DRAM to spread the
                            # 1024 values over 128 partitions so the ACT
                            # passes cost 8 columns instead of 1024
                            slot = nblk[0] % 8
                            nblk[0] += 1
                            nc.sync.dma_start(dsc_d[slot:slot + 1, :],
                                              o_sb[64:65, :, :, :])
                            dsp = bpool.tile([P, 8], bf16, tag="dsp")
                            nc.sync.dma_start(
                                dsp[:],
                                dsc_d[slot:slot + 1, :].rearrange(
                                    "o (p c) -> (o p) c", p=P, c=8))
                            dl8 = bpool.tile([P, 8], f32, tag="dl8")
                            nc.scalar.activation(dl8[:], dsp[:], AF.Ln)
                            dr8 = bpool.tile([P, 8], bf16, tag="dr8")
                            nc.scalar.activation(dr8[:], dl8[:], AF.Exp,
                                                 scale=-1.0)
                            nc.sync.dma_start(
                                dsc2_d[slot:slot + 1, :].rearrange(
                                    "o (p c) -> (o p) c", p=P, c=8), dr8[:])
                            drow = bpool.tile([1, 2, 2, TQ], bf16, tag="drow")
                            nc.sync.dma_start(drow[:], dsc2_d[slot:slot + 1, :])
                            bc2 = bpool.tile([64, 2, 2, TQ], bf16, tag="bc2")
                            nc.gpsimd.partition_broadcast(bc2[:], drow[:])

                            # the normalize mults go on DVE but DEFERRED one
                            # block, so they sit behind the next block's
                            # evacuation in the DVE queue and never delay it
                            # while the denominator DMA chain is in flight
                            def n2(o_sb=o_sb, bc2=bc2, gp=gp, qsl=qsl,
                                   orhs=orhs):
                                nc.vector.tensor_tensor(
                                    orhs[gp, 0:2, qsl], o_sb[0:64, 0, :, :],
                                    bc2[:, 0, :, :], ALU.mult)
                                nc.vector.tensor_tensor(
                                    orhs[gp, 2:4, qsl], o_sb[0:64, 1, :, :],
                                    bc2[:, 1, :, :], ALU.mult)
                            norm2.append(n2)

                        # last tb only: queue the first-half output projection
                        # before the final block (there is no later work to
                        # drain it against); all other tbs queue both halves
                        # at tb end so a drain never pops before its orhs is
                        # normalized
                        if bi == 2 and last:
                            queue_E(tb, orhs, range(NDC), half=0)
                    while norm2:
                        norm2.pop(0)()
                    if not last:
                        queue_E(tb, orhs, range(NDC), half=0)
                        queue_E(tb, orhs, range(NDC), half=1)
                    else:
                        fst = kvpool.tile([P, NDC, TQ], bf16)
                        queue_E(tb, orhs, range(NDC), half=1, fstage=fst)
                drain(len(pending))
                drain(len(pending))

    nc.compile()
    return nc


_NC_CACHE = None


def _get_nc():
    global _NC_CACHE
    if _NC_CACHE is None:
        _NC_CACHE = _build_nc()
    return _NC_CACHE


def _host_constants(q_scale, k_scale):
    pos = np.arange(T, dtype=np.float64)
    invf = 1.0 / (THETA ** (np.arange(0, HD, 2, dtype=np.float64) / HD))  # (32,)
    ang = pos[:, None] * invf[None, :]                                    # (T, 32)
    c = np.cos(ang)
    s = np.sin(ang)
    pidx = np.arange(P) % 32
    hidx = np.arange(P) % HD
    cosq = (c[:, pidx].T * q_scale[hidx][:, None]).astype(ml_dtypes.bfloat16)
    cosk = (c[:, pidx].T * k_scale[hidx][:, None]).astype(ml_dtypes.bfloat16)
    sin = s[:, pidx].T.astype(ml_dtypes.bfloat16)

    def rmat(scale):
        R = np.zeros((HD, HD), dtype=np.float64)
        for i in range(32):
            R[i, i + 32] = -scale[i + 32]
            R[i + 32, i] = scale[i]
        M = np.kron(np.eye(2), R)
        return np.ascontiguousarray(M.T).astype(ml_dtypes.bfloat16)

    hsel = np.zeros((P, 2), dtype=np.float32)
    hsel[0:64, 0] = 1.0
    hsel[64:P, 1] = 1.0
    hexp = np.ascontiguousarray(hsel.T).astype(ml_dtypes.bfloat16)
    hsel = hsel.astype(ml_dtypes.bfloat16)

    # masks[p, i, j, f] = (f >= p + 128*i), replicated over the 4 head slots
    pp = np.arange(P)[:, None]
    ff = np.arange(TQ)[None, :]
    masks = np.zeros((P, 2, GROUPS, TQ), dtype=np.float32)
    for i in range(2):
        m = (ff >= pp + P * i).astype(np.float32)
        for j in range(GROUPS):
            masks[:, i, j, :] = m
    masks = masks.astype(ml_dtypes.bfloat16)
    ident = np.eye(P, dtype=ml_dtypes.bfloat16)

    return cosq, cosk, sin, rmat(q_scale), rmat(k_scale), hsel, hexp, masks, ident


def _run(inputs, trace=False):
    x = np.asarray(inputs["x"], dtype=np.float32)
    Wq = np.asarray(inputs["Wq"], dtype=np.float32)
    Wk = np.asarray(inputs["Wk"], dtype=np.float32)
    Wv = np.asarray(inputs["Wv"], dtype=np.float32)
    Wo = np.asarray(inputs["Wo"], dtype=np.float32)
    q_scale = np.asarray(inputs["q_scale"], dtype=np.float64)
    k_scale = np.asarray(inputs["k_scale"], dtype=np.float64)

    cosq, cosk, sin, rqT, rkT, hsel, hexp, masks, ident = _host_constants(
        q_scale, k_scale)

    in_maps = []
    for cid in range(8):
        b = cid // 4
        r = cid % 4
        # Wo rows (g, head j within group, hd) -> [64g+hd partitions, j slots]
        wo_loc = Wo[r * FQ:(r + 1) * FQ, :].reshape(KVL, GROUPS, HD, D)
        wo_loc = np.ascontiguousarray(
            wo_loc.transpose(0, 2, 1, 3)).reshape(P, GROUPS, D)
        def parr(W, f0, f1):
            # [D, F] slice -> [P, NDC, F] so each SBUF partition's data is one
            # contiguous DMA line
            w = W[:, f0:f1].reshape(NDC, P, f1 - f0)
            return np.ascontiguousarray(
                w.transpose(1, 0, 2)).astype(ml_dtypes.bfloat16)

        in_maps.append({
            "xT": np.ascontiguousarray(x[b].T).astype(ml_dtypes.bfloat16),
            "wq": parr(Wq, r * FQ, (r + 1) * FQ),
            "wk": parr(Wk, r * FKV, (r + 1) * FKV),
            "wv": parr(Wv, r * FKV, (r + 1) * FKV),
            "wo": wo_loc.astype(ml_dtypes.bfloat16),
            "cosq": cosq, "cosk": cosk, "sin": sin,
            "rqT": rqT, "rkT": rkT, "hsel": hsel, "hexp": hexp,
            "masks": masks, "ident": ident,
        })

    nc = _get_nc()
    res = run_bass_kernel_spmd(nc, in_maps, core_ids=list(range(8)), trace=trace)
    out = np.empty((B, T, D), dtype=np.float32)
    for b in range(B):
        acc = res.results[4 * b]["outT"].astype(np.float32)
        accF = res.results[4 * b]["outF"].astype(np.float32)
        for r in range(1, 4):
            acc = acc + res.results[4 * b + r]["outT"].astype(np.float32)
            accF = accF + res.results[4 * b + r]["outF"].astype(np.float32)
        # outF is the packed final query-half: [P, NDC, TQ] -> [D, TQ]
        acc[:, T - TQ:T] = accF.transpose(1, 0, 2).reshape(D, TQ)
        out[b] = acc.T
    return out, res


def kernel(**inputs):
    out, _ = _run(inputs, trace=False)
    return out
